# revision 1
# baseline (speedup 1.0000x reference)
"""Trainium2 Bass kernel for nn_GroupAttentionLayer (sparse block attention).

Strategy (8 NeuronCores, SPMD):
  Query sharding: core i handles batch b=i//2, query-pixel half h=i%2
  (2048 query pixels each). Attention, Conv_K accumulator and CBL_Q are
  computed per-batch with channel-major layouts so every reduction lands
  on the natural engine axis:

    scores^T[k,q] = Qc[:,k].T @ Xq[:,q]          (PE, contract channels)
    E = exp(scores/8)                             (ACT, fused 1/8 scale)
    D_bcast = blockmap.T @ E                      (PE; per-64-block sums,
                                                   pre-broadcast over partitions)
    A = E * recip(D_bcast)                        (DVE/POOL split)
    agg^T[c,q] += x_block[k,:].T @ A              (PE, contract keys, PSUM acc,
                                                   Conv_K folded in as first matmul)

  Two collectives: AllReduce of BN_Q batch stats ([128,2]) and AllGather
  of the per-core z1 shard (1 MB/rank). The epilogue (BN1 + spatial
  softmax + CBL_O) runs redundantly on every core from the gathered
  full tensor, so no further syncs are needed.

Host side: shards/transposes inputs with numpy, assembles the full
output from core 0's channel-major result.
"""

import numpy as np

B, H, W, C = 4, 64, 64, 128
RF = 8
EPS = 1e-3
ALPHA = 0.1
N_CORES = 8
HWPIX = H * W            # 4096 pixels per batch
QSH = HWPIX * B // N_CORES  # 2048 query pixels per core
PW = W + 2               # 66, padded row width
PADN = PW * (H + 2)      # 4356 padded columns
NKT = HWPIX // 128       # 32 key tiles per batch
NQT = QSH // 512         # 4 query tiles per core
NCH = (HWPIX * B) // 512  # 32 epilogue chunks
F32 = None               # set on first build (mybir.dt.float32)

# 1 of every DVE_EVERY normalize-multiplies runs on DVE; the rest on POOL
DVE_EVERY = 3

DEBUG = False  # adds intermediate-tensor outputs for bisection

_CACHE = {}


def _build_program():
    import concourse.bacc as bacc
    import concourse.tile as tile
    from concourse import mybir

    f32 = mybir.dt.float32
    f32r = mybir.dt.float32r
    AF = mybir.ActivationFunctionType
    OP = mybir.AluOpType
    AX = mybir.AxisListType

    nc = bacc.Bacc("TRN2", target_bir_lowering=False, debug=False,
                   enable_asserts=True, num_devices=N_CORES)

    # per-core inputs
    d_xb = nc.dram_tensor("xb", [HWPIX, C], f32, kind="ExternalInput").ap()
    d_xqT = nc.dram_tensor("xqT", [C, QSH], f32, kind="ExternalInput").ap()
    d_xpadT = nc.dram_tensor("xpadT", [C, PADN], f32, kind="ExternalInput").ap()
    # shared inputs
    d_wq9 = nc.dram_tensor("wq9", [9, C, C], f32, kind="ExternalInput").ap()
    d_wk = nc.dram_tensor("wk", [C, C], f32, kind="ExternalInput").ap()
    d_wo = nc.dram_tensor("wo", [C, C], f32, kind="ExternalInput").ap()
    d_vecs = nc.dram_tensor("vecs", [6, C], f32, kind="ExternalInput").ap()
    d_bm = nc.dram_tensor("bm", [C, C], f32, kind="ExternalInput").ap()
    # output: full channel-major result (identical on every core)
    d_outT = nc.dram_tensor("outT", [C, B * HWPIX], f32, kind="ExternalOutput").ap()
    if DEBUG:
        d_dbg_qc = nc.dram_tensor("dbg_qc", [C, HWPIX], f32,
                                  kind="ExternalOutput").ap()
        d_dbg_z1 = nc.dram_tensor("dbg_z1", [C, QSH], f32,
                                  kind="ExternalOutput").ap()
        d_dbg_zfull = nc.dram_tensor("dbg_zfull", [C, B * HWPIX], f32,
                                     kind="ExternalOutput").ap()

    with tile.TileContext(nc) as tc:
        with tc.tile_pool(name="const", bufs=1) as const, \
             tc.tile_pool(name="big", bufs=1) as big, \
             tc.tile_pool(name="work", bufs=6) as work, \
             tc.tile_pool(name="tmp2", bufs=2) as tmp2p, \
             tc.tile_pool(name="zbig", bufs=1) as zbig, \
             tc.tile_pool(name="small", bufs=2) as small, \
             tc.tile_pool(name="ps", bufs=3, space="PSUM") as ps, \
             tc.tile_pool(name="psA", bufs=2, space="PSUM") as psA, \
             tc.tile_pool(name="dram", bufs=1, space="DRAM") as dram:

            # ---------------- loads ----------------
            Xpad = big.tile([C, PADN], f32r)
            nc.sync.dma_start(Xpad[:], d_xpadT[:].bitcast(f32r))
            Xq = big.tile([C, QSH], f32r)
            nc.sync.dma_start(Xq[:], d_xqT[:].bitcast(f32r))
            Xnat = big.tile([128, NKT, C], f32r)
            nc.scalar.dma_start(
                Xnat[:], d_xb.rearrange("(t p) c -> p t c", p=128).bitcast(f32r))
            Wq_s = const.tile([C, 9, C], f32r)
            nc.scalar.dma_start(
                Wq_s[:], d_wq9.rearrange("t ci co -> ci t co").bitcast(f32r))
            Wk_s = const.tile([C, C], f32r)
            nc.sync.dma_start(Wk_s[:], d_wk[:].bitcast(f32r))
            Wo_s = const.tile([C, C], f32r)
            nc.sync.dma_start(Wo_s[:], d_wo[:].bitcast(f32r))
            V = const.tile([C, 6], f32)
            nc.scalar.dma_start(V[:], d_vecs.rearrange("v c -> c v"))
            Bb = const.tile([C, C], f32r)
            nc.sync.dma_start(Bb[:], d_bm[:].bitcast(f32r))
            eps_t = const.tile([C, 1], f32)
            nc.vector.memset(eps_t[:], EPS)

            Xpv = Xpad[:].rearrange("p (r c) -> p r c", r=H + 2)

            # ---------------- CBL_Q: conv3x3 + batch stats ----------------
            # Conv output is produced directly in BLOCK-MAJOR key order:
            # chunk t covers block-row n=t, column order (m, p, q) so that
            # Qc column n*512 + m*64 + p*8 + q is pixel (8n+p, 8m+q). Each
            # 128-column slice of Qc is then two complete 8x8 blocks,
            # matching the blockmap and the host-side permutation of xb.
            Zq = zbig.tile([C, 8, 512], f32, tag="zbig",
                           padded_shape=[C, 8, B * HWPIX // 8])  # shares Z slot
            qstats = small.tile([C, 8, 6], f32)
            for t in range(8):
                pq = psA.tile([C, 512], f32, tag="agg")
                for tap in range(9):
                    dh, dw = tap // 3 - 1, tap % 3 - 1
                    rhs = Xpv[:, t * 8 + 1 + dh: t * 8 + 9 + dh,
                              1 + dw: 65 + dw].rearrange(
                                  "c p (m q) -> c m p q", m=8)
                    nc.tensor.matmul(pq[:], Wq_s[:, tap, :], rhs,
                                     start=(tap == 0), stop=(tap == 8))
                nc.vector.bn_stats(qstats[:, t, :], pq[:])
                nc.scalar.copy(Zq[:, t, :], pq[:])

            qmv = small.tile([C, 2], f32)
            nc.vector.bn_aggr(qmv[:], qstats[:])
            # partial sums for the global (cross-core) stats:
            #   sums[:,0] = mean * 4096 ; sums[:,1] = (var + mean^2) * 4096
            sums = small.tile([C, 2], f32)
            nc.vector.tensor_scalar_mul(sums[:, 0:1], qmv[:, 0:1], float(HWPIX))
            m2 = small.tile([C, 1], f32)
            nc.vector.tensor_mul(m2[:], qmv[:, 0:1], qmv[:, 0:1])
            nc.vector.tensor_add(m2[:], m2[:], qmv[:, 1:2])
            nc.vector.tensor_scalar_mul(sums[:, 1:2], m2[:], float(HWPIX))

            st_in = dram.tile([C, 2], f32)
            st_out = dram.tile([C, 2], f32, addr_space="Shared")
            nc.sync.dma_start(st_in[:], sums[:])
            nc.gpsimd.collective_compute(
                "AllReduce", mybir.AluOpType.add,
                replica_groups=[list(range(N_CORES))],
                ins=[st_in.opt()], outs=[st_out.opt()])
            gst = small.tile([C, 2], f32)
            nc.sync.dma_start(gst[:], st_out[:])

            # global mean / var (each batch appears twice in the sum)
            TOT = float(HWPIX * N_CORES)
            mean_g = small.tile([C, 1], f32)
            nc.vector.tensor_scalar_mul(mean_g[:], gst[:, 0:1], 1.0 / TOT)
            negvar = small.tile([C, 1], f32)
            # (mean*mean) - E[z^2]  ->  -var
            ez2 = small.tile([C, 1], f32)
            nc.vector.tensor_scalar_mul(ez2[:], gst[:, 1:2], 1.0 / TOT)
            nc.vector.scalar_tensor_tensor(negvar[:], mean_g[:], mean_g[:],
                                           ez2[:], op0=OP.mult,
                                           op1=OP.subtract)
            std = small.tile([C, 1], f32)
            nc.scalar.activation(std[:], negvar[:], AF.Sqrt,
                                 scale=-1.0, bias=eps_t[:])
            rstd = small.tile([C, 1], f32)
            nc.vector.reciprocal(rstd[:], std[:])
            aq = small.tile([C, 1], f32)
            nc.vector.tensor_mul(aq[:], rstd[:], V[:, 0:1])
            bq = small.tile([C, 1], f32)
            nc.vector.tensor_scalar(bq[:], mean_g[:], aq[:], -1.0,
                                    op0=OP.mult, op1=OP.mult)
            nc.vector.tensor_add(bq[:], bq[:], V[:, 1:2])

            # q = leaky(aq*z + bq); Zq is already block-major
            Qc = big.tile([C, HWPIX], f32r)
            Qv = Qc[:].rearrange("p (t f) -> p t f", f=512)
            for t in range(8):
                tmp = tmp2p.tile([C, 512], f32, tag="tmp2")
                nc.scalar.activation(tmp[:], Zq[:, t, :], AF.Identity,
                                     scale=aq[:], bias=bq[:])
                nc.vector.scalar_tensor_tensor(Qv[:, t, :], tmp[:], ALPHA,
                                               tmp[:], op0=OP.mult,
                                               op1=OP.max)

            # ---------------- attention main loop ----------------
            z1s = big.tile([C, NQT, 512], f32)
            AGW = QSH + 8
            ag_in = dram.tile([C, AGW], f32)
            ag_out = dram.tile([N_CORES * C, AGW], f32, addr_space="Shared")
            qs1 = small.tile([C, NQT, 6], f32)
            for qt in range(NQT):
                pagg = psA.tile([C, 512], f32, tag="agg")
                # Conv_K accumulator folded into the attention PSUM group
                nc.tensor.matmul(pagg[:], Wk_s[:],
                                 Xq[:, qt * 512:(qt + 1) * 512],
                                 start=True, stop=False)
                for kt in range(NKT):
                    psS = ps.tile([C, 512], f32, tag="s")
                    nc.tensor.matmul(psS[:], Qc[:, kt * 128:(kt + 1) * 128],
                                     Xq[:, qt * 512:(qt + 1) * 512],
                                     start=True, stop=True)
                    E = work.tile([C, 512], f32r, tag="E")
                    nc.scalar.activation(E[:], psS[:], AF.Exp, scale=1.0 / RF)
                    psD = ps.tile([C, 512], f32, tag="d")
                    nc.tensor.matmul(psD[:], Bb[:], E[:],
                                     start=True, stop=True)
                    R = work.tile([C, 512], f32, tag="R")
                    nc.vector.reciprocal_approx_fast(R[:], psD[:])
                    A = work.tile([C, 512], f32r, tag="A")
                    if kt % DVE_EVERY == DVE_EVERY - 1:
                        nc.vector.tensor_mul(A[:], E[:], R[:])
                    else:
                        nc.gpsimd.tensor_mul(A[:], E[:], R[:])
                    nc.tensor.matmul(pagg[:], Xnat[:, kt, :], A[:],
                                     start=False, stop=(kt == NKT - 1))
                nc.scalar.copy(z1s[:, qt, :], pagg[:])
                nc.vector.bn_stats(qs1[:, qt, :], pagg[:])
                nc.sync.dma_start(ag_in[:, qt * 512:(qt + 1) * 512],
                                  z1s[:, qt, :])

            sh_mv = small.tile([C, 2], f32)
            nc.vector.bn_aggr(sh_mv[:], qs1[:])
            sh_sums = small.tile([C, 2], f32)
            nc.vector.tensor_scalar_mul(sh_sums[:, 0:1], sh_mv[:, 0:1],
                                        float(QSH))
            shm2 = small.tile([C, 1], f32)
            nc.vector.tensor_mul(shm2[:], sh_mv[:, 0:1], sh_mv[:, 0:1])
            nc.vector.tensor_add(shm2[:], shm2[:], sh_mv[:, 1:2])
            nc.vector.tensor_scalar_mul(sh_sums[:, 1:2], shm2[:], float(QSH))
            nc.sync.dma_start(ag_in[:, QSH:QSH + 2], sh_sums[:])

            if DEBUG:
                nc.sync.dma_start(d_dbg_qc[:], Qc[:])
                nc.sync.dma_start(d_dbg_z1[:],
                                  z1s[:].rearrange("c a b -> c (a b)"))

            nc.gpsimd.collective_compute(
                "AllGather", mybir.AluOpType.bypass,
                replica_groups=[list(range(N_CORES))],
                ins=[ag_in.opt()], outs=[ag_out.opt()])

            # ---------------- epilogue (redundant on all cores) ------------
            Z = zbig.tile([C, B * HWPIX], f32, tag="zbig")
            ag_v = ag_out[:].rearrange("(r p) f -> p r f", r=N_CORES)
            Zr16 = Z[:].rearrange("p (r h f) -> p r h f", r=N_CORES, h=2)
            for r in range(N_CORES):
                for hh in range(2):
                    eng = nc.sync if (2 * r + hh) % 2 == 0 else nc.scalar
                    eng.dma_start(Zr16[:, r, hh, :],
                                  ag_v[:, r, hh * (QSH // 2):(hh + 1) * (QSH // 2)])
            Zv = Z[:].rearrange("p (t f) -> p t f", f=512)
            st8 = small.tile([C, N_CORES, 2], f32)
            nc.sync.dma_start(st8[:], ag_v[:, :, QSH:QSH + 2])
            if DEBUG:
                nc.sync.dma_start(d_dbg_zfull[:], Z[:])

            TOT1 = float(B * HWPIX)
            gsum1 = small.tile([C, 2], f32)
            nc.vector.tensor_reduce(
                gsum1[:], st8[:].rearrange("c r j -> c j r"),
                axis=AX.X, op=OP.add)
            mean1 = small.tile([C, 1], f32)
            nc.vector.tensor_scalar_mul(mean1[:], gsum1[:, 0:1], 1.0 / TOT1)
            ez21 = small.tile([C, 1], f32)
            nc.vector.tensor_scalar_mul(ez21[:], gsum1[:, 1:2], 1.0 / TOT1)
            negv1 = small.tile([C, 1], f32)
            nc.vector.scalar_tensor_tensor(negv1[:], mean1[:], mean1[:],
                                           ez21[:], op0=OP.mult,
                                           op1=OP.subtract)
            std1 = small.tile([C, 1], f32)
            nc.scalar.activation(std1[:], negv1[:], AF.Sqrt,
                                 scale=-1.0, bias=eps_t[:])
            rstd1 = small.tile([C, 1], f32)
            nc.vector.reciprocal(rstd1[:], std1[:])
            a1 = small.tile([C, 1], f32)
            nc.vector.tensor_mul(a1[:], rstd1[:], V[:, 2:3])
            b1 = small.tile([C, 1], f32)
            nc.vector.tensor_scalar(b1[:], mean1[:], a1[:], -1.0,
                                    op0=OP.mult, op1=OP.mult)
            nc.vector.tensor_add(b1[:], b1[:], V[:, 3:4])

            # exp(BN1(z)) in place, with per-chunk sums from the ACT pass
            esums = small.tile([C, NCH], f32)
            for t in range(NCH):
                nc.scalar.activation(Zv[:, t, :], Zv[:, t, :], AF.Exp,
                                     scale=a1[:], bias=b1[:],
                                     accum_out=esums[:, t:t + 1])
            rb = small.tile([C, B], f32)
            for b in range(B):
                bsum = small.tile([C, 1], f32, tag="bsum")
                nc.vector.tensor_reduce(
                    bsum[:], esums[:, b * 8:(b + 1) * 8],
                    axis=AX.X, op=OP.add)
                nc.vector.reciprocal(rb[:, b:b + 1], bsum[:])
            # CBL_O: softmax-normalize each chunk into a small fp32r
            # staging tile, 1x1 conv + batch stats, overwrite Z with the
            # conv output (the exp values are no longer needed).
            stO = big.tile([C, NCH, 6], f32)
            for t in range(NCH):
                Yt = tmp2p.tile([C, 512], f32r, tag="Yt")
                nc.vector.tensor_scalar_mul(Yt[:], Zv[:, t, :],
                                            rb[:, t // 8:t // 8 + 1])
                pzo = ps.tile([C, 512], f32, tag="s")
                nc.tensor.matmul(pzo[:], Wo_s[:], Yt[:],
                                 start=True, stop=True)
                nc.vector.bn_stats(stO[:, t, :], pzo[:])
                nc.scalar.copy(Zv[:, t, :], pzo[:])
            mvO = small.tile([C, 2], f32)
            nc.vector.bn_aggr(mvO[:], stO[:])
            stdO = small.tile([C, 1], f32)
            nc.scalar.activation(stdO[:], mvO[:, 1:2], AF.Sqrt,
                                 scale=1.0, bias=eps_t[:])
            rstdO = small.tile([C, 1], f32)
            nc.vector.reciprocal(rstdO[:], stdO[:])
            aO = small.tile([C, 1], f32)
            nc.vector.tensor_mul(aO[:], rstdO[:], V[:, 4:5])
            bO = small.tile([C, 1], f32)
            nc.vector.tensor_scalar(bO[:], mvO[:, 0:1], aO[:], -1.0,
                                    op0=OP.mult, op1=OP.mult)
            nc.vector.tensor_add(bO[:], bO[:], V[:, 5:6])

            for t in range(NCH):
                tmp = tmp2p.tile([C, 512], f32, tag="tmp2")
                nc.scalar.activation(tmp[:], Zv[:, t, :], AF.Identity,
                                     scale=aO[:], bias=bO[:])
                nc.vector.scalar_tensor_tensor(Zv[:, t, :], tmp[:], ALPHA,
                                               tmp[:], op0=OP.mult,
                                               op1=OP.max)
                eng = nc.sync if t % 2 == 0 else nc.scalar
                eng.dma_start(d_outT[:, t * 512:(t + 1) * 512],
                              Zv[:, t, :])

    nc.compile()
    return nc


def _get_runner():
    if "runner" in _CACHE:
        return _CACHE["runner"]
    import jax
    import numpy as np
    from jax.sharding import Mesh, PartitionSpec
    from jax.experimental.shard_map import shard_map
    from concourse import mybir
    from concourse.bass2jax import (_bass_exec_p, install_neuronx_cc_hook,
                                    partition_id_tensor)

    nc = _build_program()
    install_neuronx_cc_hook()

    in_names, out_names, out_avals, zero_outs = [], [], [], []
    partition_name = nc.partition_id_tensor.name if nc.partition_id_tensor else None
    for alloc in nc.m.functions[0].allocations:
        if not isinstance(alloc, mybir.MemoryLocationSet):
            continue
        name = alloc.memorylocations[0].name
        if alloc.kind == "ExternalInput":
            if name != partition_name:
                in_names.append(name)
        elif alloc.kind == "ExternalOutput":
            shape = tuple(alloc.tensor_shape)
            dtype = mybir.dt.np(alloc.dtype)
            out_names.append(name)
            out_avals.append(jax.core.ShapedArray(shape, dtype))
            zero_outs.append(np.zeros(shape, dtype))
    n_params = len(in_names)
    n_outs = len(out_avals)
    all_in_names = list(in_names) + list(out_names)
    if partition_name is not None:
        all_in_names.append(partition_name)

    def _body(*args):
        operands = list(args)
        if partition_name is not None:
            operands.append(partition_id_tensor())
        outs = _bass_exec_p.bind(
            *operands,
            out_avals=tuple(out_avals),
            in_names=tuple(all_in_names),
            out_names=tuple(out_names),
            lowering_input_output_aliases=(),
            sim_require_finite=True,
            sim_require_nnan=True,
            nc=nc,
        )
        return tuple(outs)

    donate = tuple(range(n_params, n_params + n_outs))
    try:
        devices = jax.devices("axon")[:N_CORES]
    except RuntimeError:
        devices = jax.devices()[:N_CORES]
    mesh = Mesh(np.asarray(devices), ("core",))
    in_specs = (PartitionSpec("core"),) * (n_params + n_outs)
    out_specs = (PartitionSpec("core"),) * n_outs
    sharded = jax.jit(
        shard_map(_body, mesh=mesh, in_specs=in_specs, out_specs=out_specs,
                  check_rep=False),
        donate_argnums=donate, keep_unused=True)

    def run(in_maps):
        per_core = [[np.asarray(m[name]) for name in in_names] for m in in_maps]
        concat_in = [np.concatenate([per_core[c][i] for c in range(N_CORES)],
                                    axis=0) for i in range(n_params)]
        concat_zeros = [np.zeros((N_CORES * z.shape[0], *z.shape[1:]), z.dtype)
                        for z in zero_outs]
        out_arrs = jax.block_until_ready(sharded(*concat_in, *concat_zeros))
        return [
            {name: np.asarray(out_arrs[i]).reshape(N_CORES, *out_avals[i].shape)[c]
             for i, name in enumerate(out_names)}
            for c in range(N_CORES)
        ]

    _CACHE["runner"] = run
    return run


def _make_blockmap():
    bm = np.zeros((C, C), np.float32)
    idx = np.arange(C)
    bm[(idx[:, None] // 64) == (idx[None, :] // 64)] = 1.0
    return bm


def kernel(x, Wq, bq, gq, btq, Wk, bk, g1, bt1, Wo, bo, go, bto):
    """Full inputs -> full output. Conv biases cancel inside training-mode
    BN (the mean subtraction removes any per-channel constant), so bq/bk/bo
    never enter the device program."""
    x = np.asarray(x, np.float32)
    run = _get_runner()

    wq9 = np.ascontiguousarray(
        np.asarray(Wq, np.float32).reshape(9, C, C))
    wk = np.ascontiguousarray(np.asarray(Wk, np.float32).reshape(C, C))
    wo = np.ascontiguousarray(np.asarray(Wo, np.float32).reshape(C, C))
    vecs = np.ascontiguousarray(np.stack([
        np.asarray(v, np.float32) for v in (gq, btq, g1, bt1, go, bto)]))
    bm = _make_blockmap()

    # block-major key permutation: tile kt=(n,j) holds blocks (n,2j),(n,2j+1)
    # with partition index mb*64 + p*8 + q  (see QcB view in _build_program)
    perm = np.arange(HWPIX).reshape(8, 8, 8, 8).transpose(0, 2, 1, 3).reshape(-1)

    in_maps = []
    for core in range(N_CORES):
        b, h = core // 2, core % 2
        xb = np.ascontiguousarray(x[b].reshape(HWPIX, C))
        xbT = xb.T  # [C, HWPIX]
        xqT = np.ascontiguousarray(xbT[:, h * QSH:(h + 1) * QSH])
        xpadT = np.zeros((C, H + 2, W + 2), np.float32)
        xpadT[:, 1:H + 1, 1:W + 1] = xbT.reshape(C, H, W)
        in_maps.append({
            "xb": np.ascontiguousarray(xb[perm]),
            "xqT": xqT,
            "xpadT": np.ascontiguousarray(xpadT.reshape(C, PADN)),
            "wq9": wq9, "wk": wk, "wo": wo, "vecs": vecs, "bm": bm,
        })

    res = run(in_maps)
    outT = res[0]["outT"]  # [C, B*HWPIX], identical on every core
    return np.ascontiguousarray(outT.T).reshape(B, H, W, C)



# revision 3
# speedup vs baseline: 2.2255x; 2.2255x over previous
"""Trainium2 Bass kernel for nn_GroupAttentionLayer (sparse block attention).

Strategy (8 NeuronCores, SPMD):
  Query sharding: core i handles batch b=i//2, query-pixel half h=i%2
  (2048 query pixels each). Attention, Conv_K accumulator and CBL_Q are
  computed per-batch with channel-major layouts; all matmul inputs are
  fp16 (PE runs fp16 at 1 cycle/row, same as f32r, at half the DMA and
  SBUF cost):

    scores^T[k,q] = Qc[:,k].T @ Xq[:,q]          (PE, contract channels)
    E = exp(scores/8)                             (ACT, 2-PSUM-bank units)
    D_bcast = blockmap.T @ E                      (PE; per-64-block sums,
                                                   pre-broadcast over partitions)
    A = E / D_bcast                               (single divide, DVE/GPSIMD split)
    agg^T[c,q] += x_block[k,:].T @ A              (PE, contract keys, PSUM acc,
                                                   Conv_K folded in as first matmul)

  The epilogue is fully SHARDED (each core finishes only its own 2048
  pixels); the BN/softmax global statistics travel through four tiny
  [C,2] AllGather collectives (BN_Q stats, BN1 stats, per-batch softmax
  sums, BN_O stats) instead of the fat z1 AllGather.

Host side: shards/transposes inputs with numpy (fp16 casts), assembles
the full output from the 8 per-core channel-major output shards.
"""

import numpy as np

B, H, W, C = 4, 64, 64, 128
RF = 8
EPS = 1e-3
ALPHA = 0.1
N_CORES = 8
HWPIX = H * W            # 4096 pixels per batch
QSH = HWPIX * B // N_CORES  # 2048 query pixels per core
PW = W + 2               # 66, padded row width
PADN = PW * (H + 2)      # 4356 padded columns
NKT = HWPIX // 128       # 32 key tiles per batch
NQT = QSH // 512         # 4 query tiles per core

# divide work split: DVE takes kt % 9 < DVE_RATIO, gpsimd the rest
DVE_RATIO = 5

DEBUG = False

_CACHE = {}


def _build_program():
    import concourse.bacc as bacc
    import concourse.tile as tile
    from concourse import mybir

    f32 = mybir.dt.float32
    f16 = mybir.dt.float16
    AF = mybir.ActivationFunctionType
    OP = mybir.AluOpType
    AX = mybir.AxisListType

    nc = bacc.Bacc("TRN2", target_bir_lowering=False, debug=False,
                   enable_asserts=True, num_devices=N_CORES)

    # per-core inputs (fp16 for matmul operands)
    d_xqT = nc.dram_tensor("xqT", [C, QSH], f16, kind="ExternalInput").ap()
    d_xpadT = nc.dram_tensor("xpadT", [C, PADN], f16, kind="ExternalInput").ap()
    d_xnat = nc.dram_tensor("xnat", [128, NKT, C], f16, kind="ExternalInput").ap()
    # shared inputs
    d_wq9 = nc.dram_tensor("wq9", [C, 9, C], f16, kind="ExternalInput").ap()
    d_wk = nc.dram_tensor("wk", [C, C], f16, kind="ExternalInput").ap()
    d_wo = nc.dram_tensor("wo", [C, C], f32, kind="ExternalInput").ap()
    d_vecs = nc.dram_tensor("vecs", [C, 6], f32, kind="ExternalInput").ap()
    d_bm = nc.dram_tensor("bm", [C, C], f16, kind="ExternalInput").ap()
    d_selb = nc.dram_tensor("selb", [C, N_CORES], f32, kind="ExternalInput").ap()
    # output: this core's shard, channel-major
    d_outT = nc.dram_tensor("outT", [C, QSH], f32, kind="ExternalOutput").ap()
    if DEBUG:
        d_dbg_qc = nc.dram_tensor("dbg_qc", [C, HWPIX], f32,
                                  kind="ExternalOutput").ap()
        d_dbg_z1 = nc.dram_tensor("dbg_z1", [C, QSH], f32,
                                  kind="ExternalOutput").ap()

    with tile.TileContext(nc) as tc:
        with tc.tile_pool(name="const", bufs=1) as const, \
             tc.tile_pool(name="big", bufs=1) as big, \
             tc.tile_pool(name="epool", bufs=3) as epool, \
             tc.tile_pool(name="apool", bufs=4) as apool, \
             tc.tile_pool(name="tmp2", bufs=2) as tmp2p, \
             tc.tile_pool(name="small", bufs=2) as small, \
             tc.tile_pool(name="ps2", bufs=2, space="PSUM") as ps2, \
             tc.tile_pool(name="psd", bufs=2, space="PSUM") as psd, \
             tc.tile_pool(name="psA", bufs=2, space="PSUM") as psA, \
             tc.tile_pool(name="dram", bufs=1, space="DRAM") as dram:

            # ---------------- loads ----------------
            Xpad = big.tile([C, PADN], f16)
            nc.sync.dma_start(Xpad[:], d_xpadT[:])
            Wq_s = const.tile([C, 9, C], f16)
            nc.sync.dma_start(Wq_s[:], d_wq9[:])
            Xq = big.tile([C, QSH], f16)
            nc.scalar.dma_start(Xq[:], d_xqT[:])
            Xnat = big.tile([128, NKT, C], f16)
            nc.scalar.dma_start(Xnat[:], d_xnat[:])
            Wk_s = const.tile([C, C], f16)
            nc.gpsimd.dma_start(Wk_s[:], d_wk[:])
            Wo_s = const.tile([C, C], f32)
            nc.gpsimd.dma_start(Wo_s[:], d_wo[:])
            V = const.tile([C, 6], f32)
            nc.gpsimd.dma_start(V[:], d_vecs[:])
            Bb = const.tile([C, C], f16)
            nc.gpsimd.dma_start(Bb[:], d_bm[:])
            Selb = const.tile([C, N_CORES], f32)
            nc.gpsimd.dma_start(Selb[:], d_selb[:])
            eps_t = const.tile([C, 1], f32)
            nc.vector.memset(eps_t[:], EPS)

            Xpv = Xpad[:].rearrange("p (r c) -> p r c", r=H + 2)

            # ---------------- CBL_Q: conv3x3 + batch stats ----------------
            # Conv output is produced directly in BLOCK-MAJOR key order:
            # chunk t covers block-row n=t, column order (m, p, q) so that
            # Qc column n*512 + m*64 + p*8 + q is pixel (8n+p, 8m+q).
            Zq = big.tile([C, 8, 512], f32)
            qstats = small.tile([C, 8, 6], f32)
            for t in range(8):
                pq = psA.tile([C, 512], f32, tag="agg")
                for tap in range(9):
                    dh, dw = tap // 3 - 1, tap % 3 - 1
                    rhs = Xpv[:, t * 8 + 1 + dh: t * 8 + 9 + dh,
                              1 + dw: 65 + dw].rearrange(
                                  "c p (m q) -> c m p q", m=8)
                    nc.tensor.matmul(pq[:], Wq_s[:, tap, :], rhs,
                                     start=(tap == 0), stop=(tap == 8))
                nc.vector.bn_stats(qstats[:, t, :], pq[:])
                nc.gpsimd.tensor_copy(Zq[:, t, :], pq[:])

            qmv = small.tile([C, 2], f32)
            nc.vector.bn_aggr(qmv[:], qstats[:])
            # partial sums for the global (cross-core) stats:
            #   sums[:,0] = mean * 4096 ; sums[:,1] = (var + mean^2) * 4096
            sums = small.tile([C, 2], f32)
            nc.vector.tensor_scalar_mul(sums[:, 0:1], qmv[:, 0:1], float(HWPIX))
            m2 = small.tile([C, 1], f32)
            nc.vector.tensor_mul(m2[:], qmv[:, 0:1], qmv[:, 0:1])
            nc.vector.tensor_add(m2[:], m2[:], qmv[:, 1:2])
            nc.vector.tensor_scalar_mul(sums[:, 1:2], m2[:], float(HWPIX))

            # ---- collective 1: BN_Q stats (AllGather + local sum) ----
            st_in1 = dram.tile([C, 2], f32)
            st_out1 = dram.tile([N_CORES * C, 2], f32, addr_space="Shared")
            nc.sync.dma_start(st_in1[:], sums[:])
            nc.gpsimd.collective_compute(
                "AllGather", mybir.AluOpType.bypass,
                replica_groups=[list(range(N_CORES))],
                ins=[st_in1.opt()], outs=[st_out1.opt()])
            gst1 = small.tile([C, 8, 2], f32)
            nc.sync.dma_start(
                gst1[:], st_out1[:].rearrange("(r c) s -> c r s", r=N_CORES))
            gsum = small.tile([C, 2], f32, tag="gsum1")
            nc.vector.tensor_reduce(
                gsum[:], gst1[:].rearrange("c r j -> c j r"),
                axis=AX.X, op=OP.add)

            # global mean / var (each batch appears twice in the sum)
            TOT = float(HWPIX * N_CORES)
            mean_g = small.tile([C, 1], f32)
            nc.vector.tensor_scalar_mul(mean_g[:], gsum[:, 0:1], 1.0 / TOT)
            ez2 = small.tile([C, 1], f32)
            nc.vector.tensor_scalar_mul(ez2[:], gsum[:, 1:2], 1.0 / TOT)
            negvar = small.tile([C, 1], f32)
            nc.vector.scalar_tensor_tensor(negvar[:], mean_g[:], mean_g[:],
                                           ez2[:], op0=OP.mult,
                                           op1=OP.subtract)
            std = small.tile([C, 1], f32)
            nc.scalar.activation(std[:], negvar[:], AF.Sqrt,
                                 scale=-1.0, bias=eps_t[:])
            rstd = small.tile([C, 1], f32)
            nc.vector.reciprocal(rstd[:], std[:])
            aq = small.tile([C, 1], f32)
            nc.vector.tensor_mul(aq[:], rstd[:], V[:, 0:1])
            bq = small.tile([C, 1], f32)
            nc.vector.tensor_scalar(bq[:], mean_g[:], aq[:], -1.0,
                                    op0=OP.mult, op1=OP.mult)
            nc.vector.tensor_add(bq[:], bq[:], V[:, 1:2])

            # q = leaky(aq*z + bq) in fp16, block-major
            Qc = big.tile([C, HWPIX], f16)
            Qv = Qc[:].rearrange("p (t f) -> p t f", f=512)
            for t in range(8):
                tmp = tmp2p.tile([C, 512], f16, tag="tmp2")
                nc.scalar.activation(tmp[:], Zq[:, t, :], AF.Identity,
                                     scale=aq[:], bias=bq[:])
                nc.vector.scalar_tensor_tensor(Qv[:, t, :], tmp[:], ALPHA,
                                               tmp[:], op0=OP.mult,
                                               op1=OP.max)

            # ---------------- attention main loop ----------------
            z1s = big.tile([C, NQT, 512], f32)
            qs1 = small.tile([C, NQT, 6], f32)
            for qt in range(NQT):
                pagg = psA.tile([C, 512], f32, tag="agg")
                xqs = Xq[:, qt * 512:(qt + 1) * 512]
                # Conv_K accumulator folded into the attention PSUM group
                nc.tensor.matmul(pagg[:], Wk_s[:], xqs,
                                 start=True, stop=False)
                for ku in range(NKT // 2):   # 2-kt units for the exp
                    psS = ps2.tile([128, 1024], f32, tag="s")
                    for j in range(2):
                        kt = 2 * ku + j
                        nc.tensor.matmul(psS[:, j * 512:(j + 1) * 512],
                                         Qc[:, kt * 128:(kt + 1) * 128],
                                         xqs, start=True, stop=True)
                    E = epool.tile([128, 1024], f16, tag="E")
                    nc.scalar.activation(E[:], psS[:], AF.Exp, scale=1.0 / RF)
                    for j in range(2):
                        kt = 2 * ku + j
                        psD = psd.tile([128, 512], f32, tag="d")
                        nc.tensor.matmul(psD[:], Bb[:],
                                         E[:, j * 512:(j + 1) * 512],
                                         start=True, stop=True)
                        A = apool.tile([128, 512], f16, tag="A")
                        eng = nc.vector if (kt % 9) < DVE_RATIO else nc.gpsimd
                        eng.tensor_tensor(A[:], E[:, j * 512:(j + 1) * 512],
                                          psD[:], OP.divide)
                        nc.tensor.matmul(pagg[:], Xnat[:, kt, :], A[:],
                                         start=False, stop=(kt == NKT - 1))
                nc.gpsimd.tensor_copy(z1s[:, qt, :], pagg[:])
                nc.vector.bn_stats(qs1[:, qt, :], pagg[:])

            sh_mv = small.tile([C, 2], f32)
            nc.vector.bn_aggr(sh_mv[:], qs1[:])
            sh_sums = small.tile([C, 2], f32)
            nc.vector.tensor_scalar_mul(sh_sums[:, 0:1], sh_mv[:, 0:1],
                                        float(QSH))
            shm2 = small.tile([C, 1], f32)
            nc.vector.tensor_mul(shm2[:], sh_mv[:, 0:1], sh_mv[:, 0:1])
            nc.vector.tensor_add(shm2[:], shm2[:], sh_mv[:, 1:2])
            nc.vector.tensor_scalar_mul(sh_sums[:, 1:2], shm2[:], float(QSH))

            if DEBUG:
                nc.sync.dma_start(d_dbg_qc[:], Qc[:])
                nc.sync.dma_start(d_dbg_z1[:],
                                  z1s[:].rearrange("c a b -> c (a b)"))

            # ---- collective 2: BN1 stats (shards are disjoint) ----
            st_in2 = dram.tile([C, 2], f32)
            st_out2 = dram.tile([N_CORES * C, 2], f32, addr_space="Shared")
            nc.sync.dma_start(st_in2[:], sh_sums[:])
            nc.gpsimd.collective_compute(
                "AllGather", mybir.AluOpType.bypass,
                replica_groups=[list(range(N_CORES))],
                ins=[st_in2.opt()], outs=[st_out2.opt()])
            gst2 = small.tile([C, 8, 2], f32)
            nc.sync.dma_start(
                gst2[:], st_out2[:].rearrange("(r c) s -> c r s", r=N_CORES))
            gsum2 = small.tile([C, 2], f32, tag="gsum2")
            nc.vector.tensor_reduce(
                gsum2[:], gst2[:].rearrange("c r j -> c j r"),
                axis=AX.X, op=OP.add)

            TOT1 = float(B * HWPIX)
            mean1 = small.tile([C, 1], f32)
            nc.vector.tensor_scalar_mul(mean1[:], gsum2[:, 0:1], 1.0 / TOT1)
            ez21 = small.tile([C, 1], f32)
            nc.vector.tensor_scalar_mul(ez21[:], gsum2[:, 1:2], 1.0 / TOT1)
            negv1 = small.tile([C, 1], f32)
            nc.vector.scalar_tensor_tensor(negv1[:], mean1[:], mean1[:],
                                           ez21[:], op0=OP.mult,
                                           op1=OP.subtract)
            std1 = small.tile([C, 1], f32)
            nc.scalar.activation(std1[:], negv1[:], AF.Sqrt,
                                 scale=-1.0, bias=eps_t[:])
            rstd1 = small.tile([C, 1], f32)
            nc.vector.reciprocal(rstd1[:], std1[:])
            a1 = small.tile([C, 1], f32)
            nc.vector.tensor_mul(a1[:], rstd1[:], V[:, 2:3])
            b1 = small.tile([C, 1], f32)
            nc.vector.tensor_scalar(b1[:], mean1[:], a1[:], -1.0,
                                    op0=OP.mult, op1=OP.mult)
            nc.vector.tensor_add(b1[:], b1[:], V[:, 3:4])

            # e = exp(BN1(z1)) in fp16 with per-chunk f32 sums
            Ebig = big.tile([C, NQT, 512], f16)
            esums = small.tile([C, NQT], f32)
            for t in range(NQT):
                nc.scalar.activation(Ebig[:, t, :], z1s[:, t, :], AF.Exp,
                                     scale=a1[:], bias=b1[:],
                                     accum_out=esums[:, t:t + 1])
            esh = small.tile([C, 2], f32, tag="esh")
            nc.vector.tensor_reduce(esh[:, 0:1], esums[:], axis=AX.X,
                                    op=OP.add)
            nc.vector.tensor_copy(esh[:, 1:2], esh[:, 0:1])

            # ---- collective 3: per-batch softmax sums ----
            st_in3 = dram.tile([C, 2], f32)
            st_out3 = dram.tile([N_CORES * C, 2], f32, addr_space="Shared")
            nc.sync.dma_start(st_in3[:], esh[:])
            nc.gpsimd.collective_compute(
                "AllGather", mybir.AluOpType.bypass,
                replica_groups=[list(range(N_CORES))],
                ins=[st_in3.opt()], outs=[st_out3.opt()])
            gst3 = small.tile([C, 8, 2], f32)
            nc.sync.dma_start(
                gst3[:], st_out3[:].rearrange("(r c) s -> c r s", r=N_CORES))
            # pick this core's batch (mask is host-provided): sum over the
            # two ranks holding the same batch
            selg = small.tile([C, 8], f32)
            nc.vector.tensor_mul(selg[:], gst3[:, :, 0], Selb[:])
            sb = small.tile([C, 1], f32, tag="sb")
            nc.vector.tensor_reduce(sb[:], selg[:], axis=AX.X, op=OP.add)
            rb = small.tile([C, 1], f32, tag="rb")
            nc.vector.reciprocal(rb[:], sb[:])

            # fold the softmax normalization into the CBL_O conv weights:
            # zO = Wo^T (e * r) = (Wo * r)^T e
            WoR = const.tile([C, C], f16, tag="wor")
            nc.vector.tensor_scalar_mul(WoR[:], Wo_s[:], rb[:])

            stO = small.tile([C, 4, 6], f32)
            psO = []
            for t in range(2):
                po = ps2.tile([C, 1024], f32, tag="s")
                for j in range(2):
                    nc.tensor.matmul(po[:, j * 512:(j + 1) * 512], WoR[:],
                                     Ebig[:, 2 * t + j, :],
                                     start=True, stop=True)
                    nc.vector.bn_stats(stO[:, 2 * t + j, :],
                                       po[:, j * 512:(j + 1) * 512])
                psO.append(po)
            mvO = small.tile([C, 2], f32)
            nc.vector.bn_aggr(mvO[:], stO[:])
            # partial sums over this shard (disjoint)
            oss = small.tile([C, 2], f32, tag="oss")
            nc.vector.tensor_scalar_mul(oss[:, 0:1], mvO[:, 0:1], float(QSH))
            om2 = small.tile([C, 1], f32)
            nc.vector.tensor_mul(om2[:], mvO[:, 0:1], mvO[:, 0:1])
            nc.vector.tensor_add(om2[:], om2[:], mvO[:, 1:2])
            nc.vector.tensor_scalar_mul(oss[:, 1:2], om2[:], float(QSH))

            # ---- collective 4: BN_O stats ----
            st_in4 = dram.tile([C, 2], f32)
            st_out4 = dram.tile([N_CORES * C, 2], f32, addr_space="Shared")
            nc.sync.dma_start(st_in4[:], oss[:])
            nc.gpsimd.collective_compute(
                "AllGather", mybir.AluOpType.bypass,
                replica_groups=[list(range(N_CORES))],
                ins=[st_in4.opt()], outs=[st_out4.opt()])
            gst4 = small.tile([C, 8, 2], f32)
            nc.sync.dma_start(
                gst4[:], st_out4[:].rearrange("(r c) s -> c r s", r=N_CORES))
            gsum4 = small.tile([C, 2], f32, tag="gsum4")
            nc.vector.tensor_reduce(
                gsum4[:], gst4[:].rearrange("c r j -> c j r"),
                axis=AX.X, op=OP.add)

            meanO = small.tile([C, 1], f32)
            nc.vector.tensor_scalar_mul(meanO[:], gsum4[:, 0:1], 1.0 / TOT1)
            ez2O = small.tile([C, 1], f32)
            nc.vector.tensor_scalar_mul(ez2O[:], gsum4[:, 1:2], 1.0 / TOT1)
            negvO = small.tile([C, 1], f32)
            nc.vector.scalar_tensor_tensor(negvO[:], meanO[:], meanO[:],
                                           ez2O[:], op0=OP.mult,
                                           op1=OP.subtract)
            stdO = small.tile([C, 1], f32)
            nc.scalar.activation(stdO[:], negvO[:], AF.Sqrt,
                                 scale=-1.0, bias=eps_t[:])
            rstdO = small.tile([C, 1], f32)
            nc.vector.reciprocal(rstdO[:], stdO[:])
            aO = small.tile([C, 1], f32)
            nc.vector.tensor_mul(aO[:], rstdO[:], V[:, 4:5])
            bO = small.tile([C, 1], f32)
            nc.vector.tensor_scalar(bO[:], meanO[:], aO[:], -1.0,
                                    op0=OP.mult, op1=OP.mult)
            nc.vector.tensor_add(bO[:], bO[:], V[:, 5:6])

            for t in range(2):
                tmp = tmp2p.tile([C, 1024], f32, tag="fin")
                nc.scalar.activation(tmp[:], psO[t][:], AF.Identity,
                                     scale=aO[:], bias=bO[:])
                out_t = tmp2p.tile([C, 1024], f32, tag="fin2")
                nc.vector.scalar_tensor_tensor(out_t[:], tmp[:], ALPHA,
                                               tmp[:], op0=OP.mult,
                                               op1=OP.max)
                eng = nc.sync if t % 2 == 0 else nc.scalar
                eng.dma_start(d_outT[:, t * 1024:(t + 1) * 1024], out_t[:])

    nc.compile()
    return nc


def _get_runner():
    if "runner" in _CACHE:
        return _CACHE["runner"]
    import jax
    import numpy as np
    from jax.sharding import Mesh, PartitionSpec
    from jax.experimental.shard_map import shard_map
    from concourse import mybir
    from concourse.bass2jax import (_bass_exec_p, install_neuronx_cc_hook,
                                    partition_id_tensor)

    nc = _build_program()
    install_neuronx_cc_hook()

    in_names, out_names, out_avals, zero_outs = [], [], [], []
    partition_name = nc.partition_id_tensor.name if nc.partition_id_tensor else None
    for alloc in nc.m.functions[0].allocations:
        if not isinstance(alloc, mybir.MemoryLocationSet):
            continue
        name = alloc.memorylocations[0].name
        if alloc.kind == "ExternalInput":
            if name != partition_name:
                in_names.append(name)
        elif alloc.kind == "ExternalOutput":
            shape = tuple(alloc.tensor_shape)
            dtype = mybir.dt.np(alloc.dtype)
            out_names.append(name)
            out_avals.append(jax.core.ShapedArray(shape, dtype))
            zero_outs.append(np.zeros(shape, dtype))
    n_params = len(in_names)
    n_outs = len(out_avals)
    all_in_names = list(in_names) + list(out_names)
    if partition_name is not None:
        all_in_names.append(partition_name)

    def _body(*args):
        operands = list(args)
        if partition_name is not None:
            operands.append(partition_id_tensor())
        outs = _bass_exec_p.bind(
            *operands,
            out_avals=tuple(out_avals),
            in_names=tuple(all_in_names),
            out_names=tuple(out_names),
            lowering_input_output_aliases=(),
            sim_require_finite=True,
            sim_require_nnan=True,
            nc=nc,
        )
        return tuple(outs)

    donate = tuple(range(n_params, n_params + n_outs))
    try:
        devices = jax.devices("axon")[:N_CORES]
    except RuntimeError:
        devices = jax.devices()[:N_CORES]
    mesh = Mesh(np.asarray(devices), ("core",))
    in_specs = (PartitionSpec("core"),) * (n_params + n_outs)
    out_specs = (PartitionSpec("core"),) * n_outs
    sharded = jax.jit(
        shard_map(_body, mesh=mesh, in_specs=in_specs, out_specs=out_specs,
                  check_rep=False),
        donate_argnums=donate, keep_unused=True)

    def run(in_maps):
        per_core = [[np.asarray(m[name]) for name in in_names] for m in in_maps]
        concat_in = [np.concatenate([per_core[c][i] for c in range(N_CORES)],
                                    axis=0) for i in range(n_params)]
        concat_zeros = [np.zeros((N_CORES * z.shape[0], *z.shape[1:]), z.dtype)
                        for z in zero_outs]
        out_arrs = jax.block_until_ready(sharded(*concat_in, *concat_zeros))
        return [
            {name: np.asarray(out_arrs[i]).reshape(N_CORES, *out_avals[i].shape)[c]
             for i, name in enumerate(out_names)}
            for c in range(N_CORES)
        ]

    _CACHE["runner"] = run
    return run


def _make_blockmap():
    bm = np.zeros((C, C), np.float16)
    idx = np.arange(C)
    bm[(idx[:, None] // 64) == (idx[None, :] // 64)] = 1.0
    return bm


def kernel(x, Wq, bq, gq, btq, Wk, bk, g1, bt1, Wo, bo, go, bto):
    """Full inputs -> full output. Conv biases cancel inside training-mode
    BN (the mean subtraction removes any per-channel constant), so bq/bk/bo
    never enter the device program."""
    x = np.asarray(x, np.float32)
    run = _get_runner()

    wq9 = np.ascontiguousarray(
        np.asarray(Wq, np.float16).reshape(9, C, C).transpose(1, 0, 2))
    wk = np.ascontiguousarray(np.asarray(Wk, np.float16).reshape(C, C))
    wo = np.ascontiguousarray(np.asarray(Wo, np.float32).reshape(C, C))
    vecs = np.ascontiguousarray(np.stack(
        [np.asarray(v, np.float32) for v in (gq, btq, g1, bt1, go, bto)],
        axis=1))
    bm = _make_blockmap()

    # block-major key permutation: tile kt holds blocks (t,2j),(t,2j+1)
    # with partition index mb*64 + p*8 + q
    perm = np.arange(HWPIX).reshape(8, 8, 8, 8).transpose(0, 2, 1, 3).reshape(-1)

    in_maps = []
    for core in range(N_CORES):
        b, h = core // 2, core % 2
        xb = np.ascontiguousarray(x[b].reshape(HWPIX, C))
        xbT = xb.T  # [C, HWPIX]
        xqT = np.ascontiguousarray(xbT[:, h * QSH:(h + 1) * QSH]).astype(np.float16)
        xpadT = np.zeros((C, H + 2, W + 2), np.float16)
        xpadT[:, 1:H + 1, 1:W + 1] = xbT.reshape(C, H, W).astype(np.float16)
        xnat = np.ascontiguousarray(
            xb[perm].astype(np.float16).reshape(NKT, 128, C).transpose(1, 0, 2))
        selb = np.zeros((C, N_CORES), np.float32)
        selb[:, 2 * b] = 1.0
        selb[:, 2 * b + 1] = 1.0
        in_maps.append({
            "xqT": xqT,
            "xpadT": np.ascontiguousarray(xpadT.reshape(C, PADN)),
            "xnat": xnat,
            "wq9": wq9, "wk": wk, "wo": wo, "vecs": vecs, "bm": bm,
            "selb": selb,
        })

    res = run(in_maps)
    out = np.empty((B, HWPIX, C), np.float32)
    for core in range(N_CORES):
        b, h = core // 2, core % 2
        out[b, h * QSH:(h + 1) * QSH, :] = res[core]["outT"].T
    return out.reshape(B, H, W, C)


# revision 5
# speedup vs baseline: 2.3857x; 1.0720x over previous
"""Trainium2 Bass kernel for nn_GroupAttentionLayer (sparse block attention).

Strategy (8 NeuronCores, SPMD):
  Query sharding: core i handles batch b=i//2, query-pixel half h=i%2
  (2048 query pixels each). Attention, Conv_K accumulator and CBL_Q are
  computed per-batch with channel-major layouts; all matmul inputs are
  fp16 (PE runs fp16 at 1 cycle/row, same as f32r, at half the DMA and
  SBUF cost):

    scores^T[k,q] = Qc[:,k].T @ Xq[:,q]          (PE, contract channels)
    E = exp(scores/8)                             (ACT, 2-PSUM-bank units)
    D_bcast = blockmap.T @ E                      (PE; per-64-block sums,
                                                   pre-broadcast over partitions)
    A = E / D_bcast                               (single divide, DVE/GPSIMD split)
    agg^T[c,q] += x_block[k,:].T @ A              (PE, contract keys, PSUM acc,
                                                   Conv_K folded in as first matmul)

  The attention loop is software-pipelined two units deep (scores of
  unit u issue before Bb/divide of u-1 and agg of u-2) so the in-order
  PE queue never stalls on the ACT exp or the DVE/GPSIMD divide.

  The epilogue is fully SHARDED (each core finishes only its own 2048
  pixels); the BN/softmax global statistics travel through four tiny
  [C,2] AllGather collectives (BN_Q stats, BN1 stats, per-batch softmax
  sums, BN_O stats) instead of a fat z1 AllGather. BN rstd is computed
  as exp(-0.5*ln(var+eps)) so the ACT engine stays on one table set.
  Dummy matmuls keep the PE p-state warm through the startup DMAs and
  the first collective.

Host side: shards/transposes inputs with numpy (fp16 casts), assembles
the full output from the 8 per-core channel-major output shards.
"""

import numpy as np

B, H, W, C = 4, 64, 64, 128
RF = 8
EPS = 1e-3
ALPHA = 0.1
N_CORES = 8
HWPIX = H * W            # 4096 pixels per batch
QSH = HWPIX * B // N_CORES  # 2048 query pixels per core
PW = W + 2               # 66, padded row width
PADH = 34 * PW           # rows 0..33 / 32..65 halves
NKT = HWPIX // 128       # 32 key tiles per batch
NQT = QSH // 512         # 4 query tiles per core
NU = NKT * NQT // 2      # 64 two-kt pipeline units

# divide work split: DVE takes kt % 9 < DVE_RATIO, gpsimd the rest
DVE_RATIO = 5
WARM_START = 24          # PE warm-up dummies before the conv
WARM_AR1 = 82            # PE warm-up dummies across collective 1

DEBUG = False

_CACHE = {}


def _build_program():
    import concourse.bacc as bacc
    import concourse.tile as tile
    from concourse import mybir

    f32 = mybir.dt.float32
    f16 = mybir.dt.float16
    AF = mybir.ActivationFunctionType
    OP = mybir.AluOpType
    AX = mybir.AxisListType

    nc = bacc.Bacc("TRN2", target_bir_lowering=False, debug=False,
                   enable_asserts=True, num_devices=N_CORES)

    # per-core inputs (fp16 for matmul operands)
    d_xqT = nc.dram_tensor("xqT", [C, QSH], f16, kind="ExternalInput").ap()
    d_xpadA = nc.dram_tensor("xpadA", [C, PADH], f16, kind="ExternalInput").ap()
    d_xpadB = nc.dram_tensor("xpadB", [C, PADH], f16, kind="ExternalInput").ap()
    d_xnat = nc.dram_tensor("xnat", [128, NKT, C], f16, kind="ExternalInput").ap()
    # shared inputs
    d_wq9 = nc.dram_tensor("wq9", [C, 9, C], f16, kind="ExternalInput").ap()
    d_wk = nc.dram_tensor("wk", [C, C], f16, kind="ExternalInput").ap()
    d_wo = nc.dram_tensor("wo", [C, C], f32, kind="ExternalInput").ap()
    d_vecs = nc.dram_tensor("vecs", [C, 6], f32, kind="ExternalInput").ap()
    d_bm = nc.dram_tensor("bm", [C, C], f16, kind="ExternalInput").ap()
    d_selb = nc.dram_tensor("selb", [C, N_CORES], f32, kind="ExternalInput").ap()
    # output: this core's shard, channel-major
    d_outT = nc.dram_tensor("outT", [C, QSH], f32, kind="ExternalOutput").ap()
    if DEBUG:
        d_dbg_qc = nc.dram_tensor("dbg_qc", [C, HWPIX], f32,
                                  kind="ExternalOutput").ap()
        d_dbg_z1 = nc.dram_tensor("dbg_z1", [C, QSH], f32,
                                  kind="ExternalOutput").ap()

    with tile.TileContext(nc) as tc:
        with tc.tile_pool(name="const", bufs=1) as const, \
             tc.tile_pool(name="big", bufs=1) as big, \
             tc.tile_pool(name="epool", bufs=3) as epool, \
             tc.tile_pool(name="apool", bufs=4) as apool, \
             tc.tile_pool(name="tmp2", bufs=2) as tmp2p, \
             tc.tile_pool(name="small", bufs=2) as small, \
             tc.tile_pool(name="ps2", bufs=2, space="PSUM") as ps2, \
             tc.tile_pool(name="psd", bufs=2, space="PSUM") as psd, \
             tc.tile_pool(name="psA", bufs=2, space="PSUM") as psA, \
             tc.tile_pool(name="dram", bufs=1, space="DRAM") as dram:

            # ---------------- loads ----------------
            Wq_s = const.tile([C, 9, C], f16)
            nc.sync.dma_start(Wq_s[:], d_wq9[:])
            XpadA = big.tile([C, PADH], f16)
            nc.sync.dma_start(XpadA[:], d_xpadA[:])
            XpadB = big.tile([C, PADH], f16)
            nc.sync.dma_start(XpadB[:], d_xpadB[:])
            Xq = big.tile([C, QSH], f16)
            nc.scalar.dma_start(Xq[:], d_xqT[:])
            Xnat = big.tile([128, NKT, C], f16)
            nc.scalar.dma_start(Xnat[:], d_xnat[:])
            Wk_s = const.tile([C, C], f16)
            nc.gpsimd.dma_start(Wk_s[:], d_wk[:])
            Wo_s = const.tile([C, C], f32)
            nc.gpsimd.dma_start(Wo_s[:], d_wo[:])
            V = const.tile([C, 6], f32)
            nc.gpsimd.dma_start(V[:], d_vecs[:])
            Bb = const.tile([C, C], f16)
            nc.gpsimd.dma_start(Bb[:], d_bm[:])
            Selb = const.tile([C, N_CORES], f32)
            nc.gpsimd.dma_start(Selb[:], d_selb[:])
            eps_t = const.tile([C, 1], f32)
            nc.vector.memset(eps_t[:], EPS)

            # warm-up / act-table primer
            dum_x = const.tile([128, 512], f16)
            nc.vector.memset(dum_x[:], 0.0)
            dum_s = const.tile([C, 1], f32)
            nc.scalar.activation(dum_s[:], eps_t[:], AF.Exp)

            def warm(n):
                for _ in range(n):
                    pw = psd.tile([128, 512], f32, tag="d")
                    nc.tensor.matmul(pw[:], dum_x[:, 0:128], dum_x[:],
                                     start=True, stop=True)

            warm(WARM_START)

            XpA = XpadA[:].rearrange("p (r c) -> p r c", r=34)
            XpB = XpadB[:].rearrange("p (r c) -> p r c", r=34)

            # rstd = exp(-0.5*ln(var+eps)) keeps ACT on the ln+exp table.
            def rstd_from_negvar(negvar, tag):
                lnv = small.tile([C, 1], f32, tag=tag + "ln")
                nc.scalar.activation(lnv[:], negvar[:], AF.Ln,
                                     scale=-1.0, bias=eps_t[:])
                r = small.tile([C, 1], f32, tag=tag + "r")
                nc.scalar.activation(r[:], lnv[:], AF.Exp, scale=-0.5)
                return r

            # a,b for y = a*z + b from gathered sums [C,2] (sum, sumsq)
            def bn_params(gsum, tot, gcol, bcol, tag):
                mean = small.tile([C, 1], f32, tag=tag + "m")
                nc.vector.tensor_scalar_mul(mean[:], gsum[:, 0:1], 1.0 / tot)
                ez2 = small.tile([C, 1], f32, tag=tag + "e")
                nc.vector.tensor_scalar_mul(ez2[:], gsum[:, 1:2], 1.0 / tot)
                negvar = small.tile([C, 1], f32, tag=tag + "nv")
                nc.vector.scalar_tensor_tensor(negvar[:], mean[:], mean[:],
                                               ez2[:], op0=OP.mult,
                                               op1=OP.subtract)
                r = rstd_from_negvar(negvar, tag)
                a = small.tile([C, 1], f32, tag=tag + "a")
                nc.vector.tensor_mul(a[:], r[:], V[:, gcol:gcol + 1])
                b = small.tile([C, 1], f32, tag=tag + "b")
                nc.vector.tensor_scalar(b[:], mean[:], a[:], -1.0,
                                        op0=OP.mult, op1=OP.mult)
                nc.vector.tensor_add(b[:], b[:], V[:, bcol:bcol + 1])
                return a, b

            # mean/var partial accumulators -> [C,2] (sum, sumsq) partials
            def partial_sums(mv, count, tag):
                s = small.tile([C, 2], f32, tag=tag)
                nc.vector.tensor_scalar_mul(s[:, 0:1], mv[:, 0:1], count)
                m2 = small.tile([C, 1], f32, tag=tag + "m2")
                nc.vector.tensor_mul(m2[:], mv[:, 0:1], mv[:, 0:1])
                nc.vector.tensor_add(m2[:], m2[:], mv[:, 1:2])
                nc.vector.tensor_scalar_mul(s[:, 1:2], m2[:], count)
                return s

            # AllGather of a [C,2] f32 payload; returns [C,8,2] in SBUF
            def gather_stats(payload, tag):
                st_in = dram.tile([C, 2], f32, tag=tag + "i")
                st_out = dram.tile([N_CORES * C, 2], f32, tag=tag + "o",
                                   addr_space="Shared")
                nc.sync.dma_start(st_in[:], payload[:])
                nc.gpsimd.collective_compute(
                    "AllGather", mybir.AluOpType.bypass,
                    replica_groups=[list(range(N_CORES))],
                    ins=[st_in.opt()], outs=[st_out.opt()])
                g = small.tile([C, 8, 2], f32, tag=tag + "g")
                nc.sync.dma_start(
                    g[:], st_out[:].rearrange("(r c) s -> c r s", r=N_CORES))
                return g

            def reduce_ranks(g, tag):
                s = small.tile([C, 2], f32, tag=tag)
                nc.vector.tensor_reduce(
                    s[:], g[:].rearrange("c r j -> c j r"),
                    axis=AX.X, op=OP.add)
                return s

            # ---------------- CBL_Q: conv3x3 + batch stats ----------------
            # Conv output is produced directly in BLOCK-MAJOR key order:
            # chunk t covers block-row n=t, column order (m, p, q) so that
            # Qc column n*512 + m*64 + p*8 + q is pixel (8n+p, 8m+q).
            Zq = big.tile([C, 8, 512], f32)
            qstats = small.tile([C, 8, 6], f32)
            for t in range(8):
                pq = psA.tile([C, 512], f32, tag="agg")
                base = t * 8 if t < 4 else t * 8 - 32
                Xp = XpA if t < 4 else XpB
                for tap in range(9):
                    dh, dw = tap // 3 - 1, tap % 3 - 1
                    rhs = Xp[:, base + 1 + dh: base + 9 + dh,
                             1 + dw: 65 + dw].rearrange(
                                 "c p (m q) -> c m p q", m=8)
                    nc.tensor.matmul(pq[:], Wq_s[:, tap, :], rhs,
                                     start=(tap == 0), stop=(tap == 8))
                nc.vector.bn_stats(qstats[:, t, :], pq[:])
                nc.gpsimd.tensor_copy(Zq[:, t, :], pq[:])

            qmv = small.tile([C, 2], f32)
            nc.vector.bn_aggr(qmv[:], qstats[:])
            sums1 = partial_sums(qmv, float(HWPIX), "p1")

            # ---- collective 1: BN_Q stats ----
            g1t = gather_stats(sums1, "c1")
            warm(WARM_AR1)
            gsum1 = reduce_ranks(g1t, "gsum1")
            # each batch appears twice in the gathered sum
            aq, bq = bn_params(gsum1, float(HWPIX * N_CORES), 0, 1, "q")

            # q = leaky(aq*z + bq) in fp16, block-major
            Qc = big.tile([C, HWPIX], f16)
            Qv = Qc[:].rearrange("p (t f) -> p t f", f=512)
            for t in range(8):
                tmp = tmp2p.tile([C, 512], f16, tag="tmp2")
                nc.scalar.activation(tmp[:], Zq[:, t, :], AF.Identity,
                                     scale=aq[:], bias=bq[:])
                nc.vector.scalar_tensor_tensor(Qv[:, t, :], tmp[:], ALPHA,
                                               tmp[:], op0=OP.mult,
                                               op1=OP.max)

            # ---------------- attention main loop ----------------
            # software-pipelined two units deep; unit u = key tiles
            # (2u, 2u+1) of query tile u//(NKT//2)
            z1s = big.tile([C, NQT, 512], f32)
            qs1 = small.tile([C, NQT, 6], f32)
            UPQ = NKT // 2            # units per query tile
            paggs = {}
            E_t = {}
            psD_t = {}
            A_t = {}

            def emit_scores(u):
                qt = u // UPQ
                xqs = Xq[:, qt * 512:(qt + 1) * 512]
                if u % UPQ == 0:
                    pagg = psA.tile([C, 512], f32, tag="agg")
                    nc.tensor.matmul(pagg[:], Wk_s[:], xqs,
                                     start=True, stop=False)
                    paggs[qt] = pagg
                psS = ps2.tile([128, 1024], f32, tag="s")
                for j in range(2):
                    kt = 2 * (u % UPQ) + j
                    nc.tensor.matmul(psS[:, j * 512:(j + 1) * 512],
                                     Qc[:, kt * 128:(kt + 1) * 128],
                                     xqs, start=True, stop=True)
                E = epool.tile([128, 1024], f16, tag="E")
                nc.scalar.activation(E[:], psS[:], AF.Exp, scale=1.0 / RF)
                E_t[u] = E

            def emit_bbdiv(u):
                E = E_t[u]
                ds, As = [], []
                for j in range(2):
                    kt = 2 * u + j  # global unit index -> kt within qt
                    psD = psd.tile([128, 512], f32, tag="d")
                    nc.tensor.matmul(psD[:], Bb[:],
                                     E[:, j * 512:(j + 1) * 512],
                                     start=True, stop=True)
                    A = apool.tile([128, 512], f16, tag="A")
                    eng = nc.vector if (kt % 9) < DVE_RATIO else nc.gpsimd
                    eng.tensor_tensor(A[:], E[:, j * 512:(j + 1) * 512],
                                      psD[:], OP.divide)
                    ds.append(psD)
                    As.append(A)
                psD_t[u] = ds
                A_t[u] = As

            def emit_agg(u):
                qt = u // UPQ
                pagg = paggs[qt]
                for j in range(2):
                    kt = 2 * (u % UPQ) + j
                    nc.tensor.matmul(pagg[:], Xnat[:, kt, :], A_t[u][j],
                                     start=False, stop=(kt == NKT - 1))
                del A_t[u]
                if u % UPQ == UPQ - 1:
                    nc.gpsimd.tensor_copy(z1s[:, qt, :], pagg[:])
                    nc.vector.bn_stats(qs1[:, qt, :], pagg[:])

            for u in range(NU):
                emit_scores(u)
                if u >= 1:
                    emit_bbdiv(u - 1)
                if u >= 2:
                    emit_agg(u - 2)
            emit_bbdiv(NU - 1)
            emit_agg(NU - 2)
            emit_agg(NU - 1)

            sh_mv = small.tile([C, 2], f32)
            nc.vector.bn_aggr(sh_mv[:], qs1[:])
            sums2 = partial_sums(sh_mv, float(QSH), "p2")

            if DEBUG:
                nc.sync.dma_start(d_dbg_qc[:], Qc[:])
                nc.sync.dma_start(d_dbg_z1[:],
                                  z1s[:].rearrange("c a b -> c (a b)"))

            # ---- collective 2: BN1 stats (shards are disjoint) ----
            g2t = gather_stats(sums2, "c2")
            gsum2 = reduce_ranks(g2t, "gsum2")
            TOT1 = float(B * HWPIX)
            a1, b1 = bn_params(gsum2, TOT1, 2, 3, "z")

            # e = exp(BN1(z1)) in fp16 with per-chunk f32 sums
            Ebig = big.tile([C, NQT, 512], f16)
            Ev = Ebig[:].rearrange("c t f -> c (t f)")
            z1v = z1s[:].rearrange("c t f -> c (t f)")
            esums = small.tile([C, 2], f32, tag="esums")
            for t in range(2):
                nc.scalar.activation(Ev[:, t * 1024:(t + 1) * 1024],
                                     z1v[:, t * 1024:(t + 1) * 1024], AF.Exp,
                                     scale=a1[:], bias=b1[:],
                                     accum_out=esums[:, t:t + 1])
            esh = small.tile([C, 2], f32, tag="esh")
            nc.vector.tensor_reduce(esh[:, 0:1], esums[:], axis=AX.X,
                                    op=OP.add)
            nc.vector.tensor_copy(esh[:, 1:2], esh[:, 0:1])

            # ---- collective 3: per-batch softmax sums ----
            g3t = gather_stats(esh, "c3")
            # pick this core's batch (mask is host-provided): sum over the
            # two ranks holding the same batch
            selg = small.tile([C, 8], f32)
            nc.vector.tensor_mul(selg[:], g3t[:, :, 0], Selb[:])
            sb = small.tile([C, 1], f32, tag="sb")
            nc.vector.tensor_reduce(sb[:], selg[:], axis=AX.X, op=OP.add)
            rb = small.tile([C, 1], f32, tag="rb")
            nc.vector.reciprocal(rb[:], sb[:])

            # fold the softmax normalization into the CBL_O conv weights:
            # zO = Wo^T (e * r) = (Wo * r)^T e
            WoR = const.tile([C, C], f16, tag="wor")
            nc.vector.tensor_scalar_mul(WoR[:], Wo_s[:], rb[:])

            stO = small.tile([C, 4, 6], f32)
            psO = []
            for t in range(2):
                po = ps2.tile([C, 1024], f32, tag="s")
                for j in range(2):
                    nc.tensor.matmul(po[:, j * 512:(j + 1) * 512], WoR[:],
                                     Ebig[:, 2 * t + j, :],
                                     start=True, stop=True)
                    nc.vector.bn_stats(stO[:, 2 * t + j, :],
                                       po[:, j * 512:(j + 1) * 512])
                psO.append(po)
            mvO = small.tile([C, 2], f32)
            nc.vector.bn_aggr(mvO[:], stO[:])
            sums4 = partial_sums(mvO, float(QSH), "p4")

            # ---- collective 4: BN_O stats ----
            g4t = gather_stats(sums4, "c4")
            gsum4 = reduce_ranks(g4t, "gsum4")
            aO, bO = bn_params(gsum4, TOT1, 4, 5, "o")

            for t in range(2):
                tmp = tmp2p.tile([C, 1024], f32, tag="fin")
                nc.scalar.activation(tmp[:], psO[t][:], AF.Identity,
                                     scale=aO[:], bias=bO[:])
                out_t = tmp2p.tile([C, 1024], f32, tag="fin2")
                nc.vector.scalar_tensor_tensor(out_t[:], tmp[:], ALPHA,
                                               tmp[:], op0=OP.mult,
                                               op1=OP.max)
                eng = nc.sync if t % 2 == 0 else nc.scalar
                eng.dma_start(d_outT[:, t * 1024:(t + 1) * 1024], out_t[:])

    nc.compile()
    return nc


def _get_runner():
    if "runner" in _CACHE:
        return _CACHE["runner"]
    import jax
    import numpy as np
    from jax.sharding import Mesh, PartitionSpec
    from jax.experimental.shard_map import shard_map
    from concourse import mybir
    from concourse.bass2jax import (_bass_exec_p, install_neuronx_cc_hook,
                                    partition_id_tensor)

    nc = _build_program()
    install_neuronx_cc_hook()

    in_names, out_names, out_avals, zero_outs = [], [], [], []
    partition_name = nc.partition_id_tensor.name if nc.partition_id_tensor else None
    for alloc in nc.m.functions[0].allocations:
        if not isinstance(alloc, mybir.MemoryLocationSet):
            continue
        name = alloc.memorylocations[0].name
        if alloc.kind == "ExternalInput":
            if name != partition_name:
                in_names.append(name)
        elif alloc.kind == "ExternalOutput":
            shape = tuple(alloc.tensor_shape)
            dtype = mybir.dt.np(alloc.dtype)
            out_names.append(name)
            out_avals.append(jax.core.ShapedArray(shape, dtype))
            zero_outs.append(np.zeros(shape, dtype))
    n_params = len(in_names)
    n_outs = len(out_avals)
    all_in_names = list(in_names) + list(out_names)
    if partition_name is not None:
        all_in_names.append(partition_name)

    def _body(*args):
        operands = list(args)
        if partition_name is not None:
            operands.append(partition_id_tensor())
        outs = _bass_exec_p.bind(
            *operands,
            out_avals=tuple(out_avals),
            in_names=tuple(all_in_names),
            out_names=tuple(out_names),
            lowering_input_output_aliases=(),
            sim_require_finite=True,
            sim_require_nnan=True,
            nc=nc,
        )
        return tuple(outs)

    donate = tuple(range(n_params, n_params + n_outs))
    try:
        devices = jax.devices("axon")[:N_CORES]
    except RuntimeError:
        devices = jax.devices()[:N_CORES]
    mesh = Mesh(np.asarray(devices), ("core",))
    in_specs = (PartitionSpec("core"),) * (n_params + n_outs)
    out_specs = (PartitionSpec("core"),) * n_outs
    sharded = jax.jit(
        shard_map(_body, mesh=mesh, in_specs=in_specs, out_specs=out_specs,
                  check_rep=False),
        donate_argnums=donate, keep_unused=True)

    def run(in_maps):
        per_core = [[np.asarray(m[name]) for name in in_names] for m in in_maps]
        concat_in = [np.concatenate([per_core[c][i] for c in range(N_CORES)],
                                    axis=0) for i in range(n_params)]
        concat_zeros = [np.zeros((N_CORES * z.shape[0], *z.shape[1:]), z.dtype)
                        for z in zero_outs]
        out_arrs = jax.block_until_ready(sharded(*concat_in, *concat_zeros))
        return [
            {name: np.asarray(out_arrs[i]).reshape(N_CORES, *out_avals[i].shape)[c]
             for i, name in enumerate(out_names)}
            for c in range(N_CORES)
        ]

    _CACHE["runner"] = run
    return run


def _make_blockmap():
    bm = np.zeros((C, C), np.float16)
    idx = np.arange(C)
    bm[(idx[:, None] // 64) == (idx[None, :] // 64)] = 1.0
    return bm


def kernel(x, Wq, bq, gq, btq, Wk, bk, g1, bt1, Wo, bo, go, bto):
    """Full inputs -> full output. Conv biases cancel inside training-mode
    BN (the mean subtraction removes any per-channel constant), so bq/bk/bo
    never enter the device program."""
    x = np.asarray(x, np.float32)
    run = _get_runner()

    wq9 = np.ascontiguousarray(
        np.asarray(Wq, np.float16).reshape(9, C, C).transpose(1, 0, 2))
    wk = np.ascontiguousarray(np.asarray(Wk, np.float16).reshape(C, C))
    wo = np.ascontiguousarray(np.asarray(Wo, np.float32).reshape(C, C))
    vecs = np.ascontiguousarray(np.stack(
        [np.asarray(v, np.float32) for v in (gq, btq, g1, bt1, go, bto)],
        axis=1))
    bm = _make_blockmap()

    # block-major key permutation: tile kt holds blocks (t,2j),(t,2j+1)
    # with partition index mb*64 + p*8 + q
    perm = np.arange(HWPIX).reshape(8, 8, 8, 8).transpose(0, 2, 1, 3).reshape(-1)

    in_maps = []
    for core in range(N_CORES):
        b, h = core // 2, core % 2
        xb = np.ascontiguousarray(x[b].reshape(HWPIX, C))
        xbT = xb.T  # [C, HWPIX]
        xqT = np.ascontiguousarray(xbT[:, h * QSH:(h + 1) * QSH]).astype(np.float16)
        xpadT = np.zeros((C, H + 2, W + 2), np.float16)
        xpadT[:, 1:H + 1, 1:W + 1] = xbT.reshape(C, H, W).astype(np.float16)
        xnat = np.ascontiguousarray(
            xb[perm].astype(np.float16).reshape(NKT, 128, C).transpose(1, 0, 2))
        selb = np.zeros((C, N_CORES), np.float32)
        selb[:, 2 * b] = 1.0
        selb[:, 2 * b + 1] = 1.0
        in_maps.append({
            "xqT": xqT,
            "xpadA": np.ascontiguousarray(xpadT[:, 0:34, :].reshape(C, PADH)),
            "xpadB": np.ascontiguousarray(xpadT[:, 32:66, :].reshape(C, PADH)),
            "xnat": xnat,
            "wq9": wq9, "wk": wk, "wo": wo, "vecs": vecs, "bm": bm,
            "selb": selb,
        })

    res = run(in_maps)
    out = np.empty((B, HWPIX, C), np.float32)
    for core in range(N_CORES):
        b, h = core // 2, core % 2
        out[b, h * QSH:(h + 1) * QSH, :] = res[core]["outT"].T
    return out.reshape(B, H, W, C)


# revision 10
# speedup vs baseline: 2.4456x; 1.0251x over previous
"""Trainium2 Bass kernel for nn_GroupAttentionLayer (sparse block attention).

Strategy (8 NeuronCores, SPMD):
  Query sharding: core i handles batch b=i//2, query-pixel half h=i%2
  (2048 query pixels each). Attention, Conv_K accumulator and CBL_Q are
  computed per-batch with channel-major layouts; all matmul inputs are
  fp16 (PE runs fp16 at 1 cycle/row, same as f32r, at half the DMA and
  SBUF cost):

    scores^T[k,q] = Qc[:,k].T @ Xq[:,q]          (PE, contract channels)
    E = exp(scores/8)                             (ACT, 2-PSUM-bank units)
    D_bcast = blockmap.T @ E                      (PE; per-64-block sums,
                                                   pre-broadcast over partitions)
    A = E / D_bcast                               (single divide, DVE/GPSIMD split)
    agg^T[c,q] += x_block[k,:].T @ A              (PE, contract keys, PSUM acc,
                                                   Conv_K folded in as first matmul)

  The attention loop is software-pipelined two units deep (scores of
  unit u issue before Bb/divide of u-1 and agg of u-2) so the in-order
  PE queue never stalls on the ACT exp or the DVE/GPSIMD divide.

  The epilogue is fully SHARDED (each core finishes only its own 2048
  pixels); the BN/softmax global statistics travel through four tiny
  [C,2] AllGather collectives (BN_Q stats, BN1 stats, per-batch softmax
  sums, BN_O stats) instead of a fat z1 AllGather. BN rstd is computed
  as exp(-0.5*ln(var+eps)) so the ACT engine stays on one table set.
  Dummy matmuls keep the PE p-state warm through the startup DMAs and
  the first collective.

Host side: shards/transposes inputs with numpy (fp16 casts), assembles
the full output from the 8 per-core channel-major output shards.
"""

import numpy as np

B, H, W, C = 4, 64, 64, 128
RF = 8
EPS = 1e-3
ALPHA = 0.1
N_CORES = 8
HWPIX = H * W            # 4096 pixels per batch
QSH = HWPIX * B // N_CORES  # 2048 query pixels per core
PW = W + 2               # 66, padded row width
PADH = 34 * PW           # rows 0..33 / 32..65 halves
NKT = HWPIX // 128       # 32 key tiles per batch
NQT = QSH // 512         # 4 query tiles per core
NU = NKT * NQT // 2      # 64 two-kt pipeline units

# divide work split: DVE takes kt % 9 < DVE_RATIO, gpsimd the rest
DVE_RATIO = 5
WARM_START = 24          # PE warm-up dummies before the conv
WARM_AR1 = 108           # PE warm-up dummies across collective 1

DEBUG = False

_CACHE = {}


def _build_program():
    import concourse.bacc as bacc
    import concourse.tile as tile
    from concourse import mybir

    f32 = mybir.dt.float32
    f16 = mybir.dt.float16
    AF = mybir.ActivationFunctionType
    OP = mybir.AluOpType
    AX = mybir.AxisListType

    nc = bacc.Bacc("TRN2", target_bir_lowering=False, debug=False,
                   enable_asserts=True, num_devices=N_CORES)

    # per-core inputs (fp16 for matmul operands)
    d_xqT = nc.dram_tensor("xqT", [C, QSH], f16, kind="ExternalInput").ap()
    d_xpadA = nc.dram_tensor("xpadA", [C, PADH], f16, kind="ExternalInput").ap()
    d_xpadB = nc.dram_tensor("xpadB", [C, PADH], f16, kind="ExternalInput").ap()
    d_xnat = nc.dram_tensor("xnat", [128, NKT, C], f16, kind="ExternalInput").ap()
    # shared inputs
    d_wq9 = nc.dram_tensor("wq9", [C, 9, C], f16, kind="ExternalInput").ap()
    d_wk = nc.dram_tensor("wk", [C, C], f16, kind="ExternalInput").ap()
    d_wo = nc.dram_tensor("wo", [C, C], f32, kind="ExternalInput").ap()
    d_vecs = nc.dram_tensor("vecs", [C, 6], f32, kind="ExternalInput").ap()
    d_bm = nc.dram_tensor("bm", [C, C], f16, kind="ExternalInput").ap()
    d_selb = nc.dram_tensor("selb", [C, N_CORES], f32, kind="ExternalInput").ap()
    # output: this core's shard, channel-major
    d_outT = nc.dram_tensor("outT", [C, QSH], f32, kind="ExternalOutput").ap()
    if DEBUG:
        d_dbg_qc = nc.dram_tensor("dbg_qc", [C, HWPIX], f32,
                                  kind="ExternalOutput").ap()
        d_dbg_z1 = nc.dram_tensor("dbg_z1", [C, QSH], f32,
                                  kind="ExternalOutput").ap()

    with tile.TileContext(nc) as tc:
        with tc.tile_pool(name="const", bufs=1) as const, \
             tc.tile_pool(name="big", bufs=1) as big, \
             tc.tile_pool(name="epool", bufs=3) as epool, \
             tc.tile_pool(name="apool", bufs=6) as apool, \
             tc.tile_pool(name="tmp2", bufs=2) as tmp2p, \
             tc.tile_pool(name="small", bufs=2) as small, \
             tc.tile_pool(name="ps2", bufs=2, space="PSUM") as ps2, \
             tc.tile_pool(name="psd", bufs=2, space="PSUM") as psd, \
             tc.tile_pool(name="psA", bufs=2, space="PSUM") as psA, \
             tc.tile_pool(name="dram", bufs=1, space="DRAM") as dram:

            # ---------------- loads ----------------
            Wq_s = const.tile([C, 9, C], f16)
            nc.sync.dma_start(Wq_s[:], d_wq9[:])
            XpadA = big.tile([C, PADH], f16)
            nc.sync.dma_start(XpadA[:], d_xpadA[:])
            XpadB = big.tile([C, PADH], f16)
            nc.sync.dma_start(XpadB[:], d_xpadB[:])
            Xq = big.tile([C, QSH], f16)
            nc.scalar.dma_start(Xq[:], d_xqT[:])
            Xnat = big.tile([128, NKT, C], f16)
            nc.scalar.dma_start(Xnat[:], d_xnat[:])
            Wk_s = const.tile([C, C], f16)
            nc.gpsimd.dma_start(Wk_s[:], d_wk[:])
            Wo_s = const.tile([C, C], f32)
            nc.gpsimd.dma_start(Wo_s[:], d_wo[:])
            V = const.tile([C, 6], f32)
            nc.gpsimd.dma_start(V[:], d_vecs[:])
            Bb = const.tile([C, C], f16)
            nc.gpsimd.dma_start(Bb[:], d_bm[:])
            Selb = const.tile([C, N_CORES], f32)
            nc.gpsimd.dma_start(Selb[:], d_selb[:])
            eps_t = const.tile([C, 1], f32)
            nc.vector.memset(eps_t[:], EPS)

            # warm-up / act-table primer: load the one table set that holds
            # every activation this program uses (exp, ln, identity, copy)
            # up front so no mid-program table switches are ever needed.
            from concourse.hw_specs import get_activation_tables
            act_sets = list(get_activation_tables(nc.m.arch).items())
            want = next((i for i, (_, fns) in enumerate(act_sets)
                         if AF.Exp in fns and AF.Ln in fns), None)
            if want is not None:
                nc.scalar.add_instruction(mybir.InstLoadActFuncSet(
                    name=nc.get_next_instruction_name(),
                    act_func_set_id=want, ins=[], outs=[]))
            dum_x = const.tile([128, 512], f16)
            nc.vector.memset(dum_x[:], 0.0)

            def warm(n):
                for _ in range(n):
                    pw = psd.tile([128, 512], f32, tag="d")
                    nc.tensor.matmul(pw[:], dum_x[:, 0:128], dum_x[:],
                                     start=True, stop=True)

            warm(WARM_START)

            XpA = XpadA[:].rearrange("p (r c) -> p r c", r=34)
            XpB = XpadB[:].rearrange("p (r c) -> p r c", r=34)

            # rstd = exp(-0.5*ln(var+eps)) keeps ACT on the ln+exp table.
            def rstd_from_negvar(negvar, tag):
                lnv = small.tile([C, 1], f32, tag=tag + "ln")
                nc.scalar.activation(lnv[:], negvar[:], AF.Ln,
                                     scale=-1.0, bias=eps_t[:])
                r = small.tile([C, 1], f32, tag=tag + "r")
                nc.scalar.activation(r[:], lnv[:], AF.Exp, scale=-0.5)
                return r

            # a,b for y = a*z + b from gathered sums [C,2] (sum, sumsq)
            def bn_params(gsum, tot, gcol, bcol, tag):
                ms = small.tile([C, 2], f32, tag=tag + "ms")
                nc.vector.tensor_scalar_mul(ms[:], gsum[:], 1.0 / tot)
                negvar = small.tile([C, 1], f32, tag=tag + "nv")
                nc.vector.scalar_tensor_tensor(negvar[:], ms[:, 0:1],
                                               ms[:, 0:1], ms[:, 1:2],
                                               op0=OP.mult, op1=OP.subtract)
                r = rstd_from_negvar(negvar, tag)
                a = small.tile([C, 1], f32, tag=tag + "a")
                nc.vector.tensor_mul(a[:], r[:], V[:, gcol:gcol + 1])
                b = small.tile([C, 1], f32, tag=tag + "b")
                nc.vector.tensor_scalar(b[:], ms[:, 0:1], a[:], -1.0,
                                        op0=OP.mult, op1=OP.mult)
                nc.vector.tensor_add(b[:], b[:], V[:, bcol:bcol + 1])
                return a, b

            # mean/var partial accumulators -> [C,2] (sum, sumsq) partials
            def partial_sums(mv, count, tag):
                s = small.tile([C, 2], f32, tag=tag)
                nc.vector.tensor_scalar_mul(s[:, 0:1], mv[:, 0:1], count)
                m2 = small.tile([C, 1], f32, tag=tag + "m2")
                nc.vector.tensor_mul(m2[:], mv[:, 0:1], mv[:, 0:1])
                nc.vector.tensor_add(m2[:], m2[:], mv[:, 1:2])
                nc.vector.tensor_scalar_mul(s[:, 1:2], m2[:], count)
                return s

            # AllGather of a [C,2] f32 payload; returns [C,8,2] in SBUF
            def gather_stats(payload, tag):
                st_in = dram.tile([C, 2], f32, tag=tag + "i")
                st_out = dram.tile([N_CORES * C, 2], f32, tag=tag + "o",
                                   addr_space="Shared")
                nc.sync.dma_start(st_in[:], payload[:])
                nc.gpsimd.collective_compute(
                    "AllGather", mybir.AluOpType.bypass,
                    replica_groups=[list(range(N_CORES))],
                    ins=[st_in.opt()], outs=[st_out.opt()])
                g = small.tile([C, 8, 2], f32, tag=tag + "g")
                nc.sync.dma_start(
                    g[:], st_out[:].rearrange("(r c) s -> c r s", r=N_CORES))
                return g

            def reduce_ranks(g, tag):
                s = small.tile([C, 2], f32, tag=tag)
                nc.vector.tensor_reduce(
                    s[:], g[:].rearrange("c r j -> c j r"),
                    axis=AX.X, op=OP.add)
                return s

            # ---------------- CBL_Q: conv3x3 + batch stats ----------------
            # Conv output is produced directly in BLOCK-MAJOR key order:
            # chunk t covers block-row n=t, column order (m, p, q) so that
            # Qc column n*512 + m*64 + p*8 + q is pixel (8n+p, 8m+q).
            Zq = big.tile([C, 8, 512], f32)
            qstats = small.tile([C, 8, 6], f32)
            for t in range(8):
                pq = psA.tile([C, 512], f32, tag="agg")
                base = t * 8 if t < 4 else t * 8 - 32
                Xp = XpA if t < 4 else XpB
                for tap in range(9):
                    dh, dw = tap // 3 - 1, tap % 3 - 1
                    rhs = Xp[:, base + 1 + dh: base + 9 + dh,
                             1 + dw: 65 + dw].rearrange(
                                 "c p (m q) -> c m p q", m=8)
                    nc.tensor.matmul(pq[:], Wq_s[:, tap, :], rhs,
                                     start=(tap == 0), stop=(tap == 8))
                nc.vector.bn_stats(qstats[:, t, :], pq[:])
                nc.gpsimd.tensor_copy(Zq[:, t, :], pq[:])

            qmv = small.tile([C, 2], f32)
            nc.vector.bn_aggr(qmv[:], qstats[:])
            sums1 = partial_sums(qmv, float(HWPIX), "p1")

            # ---- collective 1: BN_Q stats ----
            g1t = gather_stats(sums1, "c1")
            warm(WARM_AR1)
            gsum1 = reduce_ranks(g1t, "gsum1")
            # each batch appears twice in the gathered sum
            aq, bq = bn_params(gsum1, float(HWPIX * N_CORES), 0, 1, "q")

            # q = leaky(aq*z + bq) in fp16, block-major
            Qc = big.tile([C, HWPIX], f16)
            Qv = Qc[:].rearrange("p (t f) -> p t f", f=512)
            for t in range(8):
                tmp = tmp2p.tile([C, 512], f16, tag="tmp2")
                nc.scalar.activation(tmp[:], Zq[:, t, :], AF.Identity,
                                     scale=aq[:], bias=bq[:])
                nc.vector.scalar_tensor_tensor(Qv[:, t, :], tmp[:], ALPHA,
                                               tmp[:], op0=OP.mult,
                                               op1=OP.max)

            # ---------------- attention main loop ----------------
            # software-pipelined two units deep; unit u = key tiles
            # (2u, 2u+1) of query tile u//(NKT//2)
            z1s = big.tile([C, NQT, 512], f32)
            qs1 = small.tile([C, NQT, 6], f32)
            UPQ = NKT // 2            # units per query tile
            paggs = {}
            E_t = {}
            psD_t = {}
            A_t = {}

            def emit_scores(u):
                qt = u // UPQ
                xqs = Xq[:, qt * 512:(qt + 1) * 512]
                if u % UPQ == 0:
                    pagg = psA.tile([C, 512], f32, tag="agg")
                    nc.tensor.matmul(pagg[:], Wk_s[:], xqs,
                                     start=True, stop=False)
                    paggs[qt] = pagg
                psS = ps2.tile([128, 1024], f32, tag="s")
                for j in range(2):
                    kt = 2 * (u % UPQ) + j
                    nc.tensor.matmul(psS[:, j * 512:(j + 1) * 512],
                                     Qc[:, kt * 128:(kt + 1) * 128],
                                     xqs, start=True, stop=True)
                E = epool.tile([128, 1024], f16, tag="E")
                nc.scalar.activation(E[:], psS[:], AF.Exp, scale=1.0 / RF)
                E_t[u] = E

            def emit_bbdiv(u):
                E = E_t[u]
                ds, As = [], []
                for j in range(2):
                    kt = 2 * u + j  # global unit index -> kt within qt
                    psD = psd.tile([128, 512], f32, tag="d")
                    nc.tensor.matmul(psD[:], Bb[:],
                                     E[:, j * 512:(j + 1) * 512],
                                     start=True, stop=True)
                    A = apool.tile([128, 512], f16, tag="A")
                    eng = nc.vector if (kt % 9) < DVE_RATIO else nc.gpsimd
                    eng.tensor_tensor(A[:], E[:, j * 512:(j + 1) * 512],
                                      psD[:], OP.divide)
                    ds.append(psD)
                    As.append(A)
                psD_t[u] = ds
                A_t[u] = As

            def emit_agg(u):
                qt = u // UPQ
                pagg = paggs[qt]
                for j in range(2):
                    kt = 2 * (u % UPQ) + j
                    nc.tensor.matmul(pagg[:], Xnat[:, kt, :], A_t[u][j],
                                     start=False, stop=(kt == NKT - 1))
                del A_t[u]
                if u % UPQ == UPQ - 1:
                    nc.gpsimd.tensor_copy(z1s[:, qt, :], pagg[:])
                    nc.vector.bn_stats(qs1[:, qt, :], pagg[:])

            for u in range(NU):
                emit_scores(u)
                if u >= 1:
                    emit_bbdiv(u - 1)
                if u >= 3:
                    emit_agg(u - 3)
            emit_bbdiv(NU - 1)
            for u in range(NU - 3, NU):
                emit_agg(u)

            sh_mv = small.tile([C, 2], f32)
            nc.vector.bn_aggr(sh_mv[:], qs1[:])
            sums2 = partial_sums(sh_mv, float(QSH), "p2")

            if DEBUG:
                nc.sync.dma_start(d_dbg_qc[:], Qc[:])
                nc.sync.dma_start(d_dbg_z1[:],
                                  z1s[:].rearrange("c a b -> c (a b)"))

            # ---- collective 2: BN1 stats (shards are disjoint) ----
            g2t = gather_stats(sums2, "c2")
            gsum2 = reduce_ranks(g2t, "gsum2")
            TOT1 = float(B * HWPIX)
            a1, b1 = bn_params(gsum2, TOT1, 2, 3, "z")

            # e = exp(BN1(z1)) in fp16 with per-chunk f32 sums
            Ebig = big.tile([C, NQT, 512], f16)
            Ev = Ebig[:].rearrange("c t f -> c (t f)")
            z1v = z1s[:].rearrange("c t f -> c (t f)")
            esums = small.tile([C, 2], f32, tag="esums")
            for t in range(2):
                nc.scalar.activation(Ev[:, t * 1024:(t + 1) * 1024],
                                     z1v[:, t * 1024:(t + 1) * 1024], AF.Exp,
                                     scale=a1[:], bias=b1[:],
                                     accum_out=esums[:, t:t + 1])
            esh = small.tile([C, 2], f32, tag="esh")
            nc.vector.tensor_reduce(esh[:, 0:1], esums[:], axis=AX.X,
                                    op=OP.add)
            nc.vector.tensor_copy(esh[:, 1:2], esh[:, 0:1])

            # ---- collective 3: per-batch softmax sums ----
            g3t = gather_stats(esh, "c3")
            # pick this core's batch (mask is host-provided): sum over the
            # two ranks holding the same batch
            selg = small.tile([C, 8], f32)
            nc.vector.tensor_mul(selg[:], g3t[:, :, 0], Selb[:])
            sb = small.tile([C, 1], f32, tag="sb")
            nc.vector.tensor_reduce(sb[:], selg[:], axis=AX.X, op=OP.add)
            rb = small.tile([C, 1], f32, tag="rb")
            nc.vector.reciprocal(rb[:], sb[:])

            # fold the softmax normalization into the CBL_O conv weights:
            # zO = Wo^T (e * r) = (Wo * r)^T e
            WoR = const.tile([C, C], f16, tag="wor")
            nc.vector.tensor_scalar_mul(WoR[:], Wo_s[:], rb[:])

            stO = small.tile([C, 4, 6], f32)
            psO = []
            for t in range(2):
                po = ps2.tile([C, 1024], f32, tag="s")
                for j in range(2):
                    nc.tensor.matmul(po[:, j * 512:(j + 1) * 512], WoR[:],
                                     Ebig[:, 2 * t + j, :],
                                     start=True, stop=True)
                    nc.vector.bn_stats(stO[:, 2 * t + j, :],
                                       po[:, j * 512:(j + 1) * 512])
                psO.append(po)
            mvO = small.tile([C, 2], f32)
            nc.vector.bn_aggr(mvO[:], stO[:])
            sums4 = partial_sums(mvO, float(QSH), "p4")

            # ---- collective 4: BN_O stats ----
            g4t = gather_stats(sums4, "c4")
            gsum4 = reduce_ranks(g4t, "gsum4")
            aO, bO = bn_params(gsum4, TOT1, 4, 5, "o")

            for t in range(2):
                tmp = tmp2p.tile([C, 1024], f32, tag="fin")
                nc.scalar.activation(tmp[:], psO[t][:], AF.Identity,
                                     scale=aO[:], bias=bO[:])
                out_t = tmp2p.tile([C, 1024], f32, tag="fin2")
                nc.vector.scalar_tensor_tensor(out_t[:], tmp[:], ALPHA,
                                               tmp[:], op0=OP.mult,
                                               op1=OP.max)
                eng = nc.sync if t % 2 == 0 else nc.scalar
                eng.dma_start(d_outT[:, t * 1024:(t + 1) * 1024], out_t[:])

    nc.compile()
    return nc


def _get_runner():
    if "runner" in _CACHE:
        return _CACHE["runner"]
    import jax
    import numpy as np
    from jax.sharding import Mesh, PartitionSpec
    from jax.experimental.shard_map import shard_map
    from concourse import mybir
    from concourse.bass2jax import (_bass_exec_p, install_neuronx_cc_hook,
                                    partition_id_tensor)

    nc = _build_program()
    install_neuronx_cc_hook()

    in_names, out_names, out_avals, zero_outs = [], [], [], []
    partition_name = nc.partition_id_tensor.name if nc.partition_id_tensor else None
    for alloc in nc.m.functions[0].allocations:
        if not isinstance(alloc, mybir.MemoryLocationSet):
            continue
        name = alloc.memorylocations[0].name
        if alloc.kind == "ExternalInput":
            if name != partition_name:
                in_names.append(name)
        elif alloc.kind == "ExternalOutput":
            shape = tuple(alloc.tensor_shape)
            dtype = mybir.dt.np(alloc.dtype)
            out_names.append(name)
            out_avals.append(jax.core.ShapedArray(shape, dtype))
            zero_outs.append(np.zeros(shape, dtype))
    n_params = len(in_names)
    n_outs = len(out_avals)
    all_in_names = list(in_names) + list(out_names)
    if partition_name is not None:
        all_in_names.append(partition_name)

    def _body(*args):
        operands = list(args)
        if partition_name is not None:
            operands.append(partition_id_tensor())
        outs = _bass_exec_p.bind(
            *operands,
            out_avals=tuple(out_avals),
            in_names=tuple(all_in_names),
            out_names=tuple(out_names),
            lowering_input_output_aliases=(),
            sim_require_finite=True,
            sim_require_nnan=True,
            nc=nc,
        )
        return tuple(outs)

    donate = tuple(range(n_params, n_params + n_outs))
    try:
        devices = jax.devices("axon")[:N_CORES]
    except RuntimeError:
        devices = jax.devices()[:N_CORES]
    mesh = Mesh(np.asarray(devices), ("core",))
    in_specs = (PartitionSpec("core"),) * (n_params + n_outs)
    out_specs = (PartitionSpec("core"),) * n_outs
    sharded = jax.jit(
        shard_map(_body, mesh=mesh, in_specs=in_specs, out_specs=out_specs,
                  check_rep=False),
        donate_argnums=donate, keep_unused=True)

    def run(in_maps):
        per_core = [[np.asarray(m[name]) for name in in_names] for m in in_maps]
        concat_in = [np.concatenate([per_core[c][i] for c in range(N_CORES)],
                                    axis=0) for i in range(n_params)]
        concat_zeros = [np.zeros((N_CORES * z.shape[0], *z.shape[1:]), z.dtype)
                        for z in zero_outs]
        out_arrs = jax.block_until_ready(sharded(*concat_in, *concat_zeros))
        return [
            {name: np.asarray(out_arrs[i]).reshape(N_CORES, *out_avals[i].shape)[c]
             for i, name in enumerate(out_names)}
            for c in range(N_CORES)
        ]

    _CACHE["runner"] = run
    return run


def _make_blockmap():
    bm = np.zeros((C, C), np.float16)
    idx = np.arange(C)
    bm[(idx[:, None] // 64) == (idx[None, :] // 64)] = 1.0
    return bm


def kernel(x, Wq, bq, gq, btq, Wk, bk, g1, bt1, Wo, bo, go, bto):
    """Full inputs -> full output. Conv biases cancel inside training-mode
    BN (the mean subtraction removes any per-channel constant), so bq/bk/bo
    never enter the device program."""
    x = np.asarray(x, np.float32)
    run = _get_runner()

    wq9 = np.ascontiguousarray(
        np.asarray(Wq, np.float16).reshape(9, C, C).transpose(1, 0, 2))
    wk = np.ascontiguousarray(np.asarray(Wk, np.float16).reshape(C, C))
    wo = np.ascontiguousarray(np.asarray(Wo, np.float32).reshape(C, C))
    vecs = np.ascontiguousarray(np.stack(
        [np.asarray(v, np.float32) for v in (gq, btq, g1, bt1, go, bto)],
        axis=1))
    bm = _make_blockmap()

    # block-major key permutation: tile kt holds blocks (t,2j),(t,2j+1)
    # with partition index mb*64 + p*8 + q
    perm = np.arange(HWPIX).reshape(8, 8, 8, 8).transpose(0, 2, 1, 3).reshape(-1)

    in_maps = []
    for core in range(N_CORES):
        b, h = core // 2, core % 2
        xb = np.ascontiguousarray(x[b].reshape(HWPIX, C))
        xbT = xb.T  # [C, HWPIX]
        xqT = np.ascontiguousarray(xbT[:, h * QSH:(h + 1) * QSH]).astype(np.float16)
        xpadT = np.zeros((C, H + 2, W + 2), np.float16)
        xpadT[:, 1:H + 1, 1:W + 1] = xbT.reshape(C, H, W).astype(np.float16)
        xnat = np.ascontiguousarray(
            xb[perm].astype(np.float16).reshape(NKT, 128, C).transpose(1, 0, 2))
        selb = np.zeros((C, N_CORES), np.float32)
        selb[:, 2 * b] = 1.0
        selb[:, 2 * b + 1] = 1.0
        in_maps.append({
            "xqT": xqT,
            "xpadA": np.ascontiguousarray(xpadT[:, 0:34, :].reshape(C, PADH)),
            "xpadB": np.ascontiguousarray(xpadT[:, 32:66, :].reshape(C, PADH)),
            "xnat": xnat,
            "wq9": wq9, "wk": wk, "wo": wo, "vecs": vecs, "bm": bm,
            "selb": selb,
        })

    res = run(in_maps)
    out = np.empty((B, HWPIX, C), np.float32)
    for core in range(N_CORES):
        b, h = core // 2, core % 2
        out[b, h * QSH:(h + 1) * QSH, :] = res[core]["outT"].T
    return out.reshape(B, H, W, C)


# revision 11
# speedup vs baseline: 2.4854x; 1.0163x over previous
"""Trainium2 Bass kernel for nn_GroupAttentionLayer (sparse block attention).

Strategy (8 NeuronCores, SPMD):
  Query sharding: core i handles batch b=i//2, query-pixel half h=i%2
  (2048 query pixels each). Attention, Conv_K accumulator and CBL_Q are
  computed per-batch with channel-major layouts; all matmul inputs are
  fp16 (PE runs fp16 at 1 cycle/row, same as f32r, at half the DMA and
  SBUF cost):

    scores^T[k,q] = Qc[:,k].T @ Xq[:,q]          (PE, contract channels)
    E = exp(scores/8)                             (ACT, 2-PSUM-bank units)
    D_bcast = blockmap.T @ E                      (PE; per-64-block sums,
                                                   pre-broadcast over partitions)
    A = E / D_bcast                               (single divide, DVE/GPSIMD split)
    agg^T[c,q] += x_block[k,:].T @ A              (PE, contract keys, PSUM acc,
                                                   Conv_K folded in as first matmul)

  The attention loop is software-pipelined two units deep (scores of
  unit u issue before Bb/divide of u-1 and agg of u-2) so the in-order
  PE queue never stalls on the ACT exp or the DVE/GPSIMD divide.

  The epilogue is fully SHARDED (each core finishes only its own 2048
  pixels); the BN/softmax global statistics travel through four tiny
  [C,2] AllGather collectives (BN_Q stats, BN1 stats, per-batch softmax
  sums, BN_O stats) instead of a fat z1 AllGather. BN rstd is computed
  as exp(-0.5*ln(var+eps)) so the ACT engine stays on one table set.
  Dummy matmuls keep the PE p-state warm through the startup DMAs and
  the first collective.

Host side: shards/transposes inputs with numpy (fp16 casts), assembles
the full output from the 8 per-core channel-major output shards.
"""

import numpy as np

B, H, W, C = 4, 64, 64, 128
RF = 8
EPS = 1e-3
ALPHA = 0.1
N_CORES = 8
HWPIX = H * W            # 4096 pixels per batch
QSH = HWPIX * B // N_CORES  # 2048 query pixels per core
PW = W + 2               # 66, padded row width
PADH = 34 * PW           # rows 0..33 / 32..65 halves
NKT = HWPIX // 128       # 32 key tiles per batch
NQT = QSH // 512         # 4 query tiles per core
NU = NKT * NQT // 2      # 64 two-kt pipeline units

# divide work split: DVE takes kt % 9 < DVE_RATIO, gpsimd the rest
DVE_RATIO = 5
WARM_START = 24          # PE warm-up dummies before the conv
WARM_AR1 = 108           # PE warm-up dummies across collective 1

DEBUG = False

_CACHE = {}


def _build_program():
    import concourse.bacc as bacc
    import concourse.tile as tile
    from concourse import mybir

    f32 = mybir.dt.float32
    f16 = mybir.dt.float16
    AF = mybir.ActivationFunctionType
    OP = mybir.AluOpType
    AX = mybir.AxisListType

    nc = bacc.Bacc("TRN2", target_bir_lowering=False, debug=False,
                   enable_asserts=True, num_devices=N_CORES)

    # per-core inputs (fp16 for matmul operands)
    d_xqT = nc.dram_tensor("xqT", [C, QSH], f16, kind="ExternalInput").ap()
    d_xpadA = nc.dram_tensor("xpadA", [C, PADH], f16, kind="ExternalInput").ap()
    d_xpadB = nc.dram_tensor("xpadB", [C, PADH], f16, kind="ExternalInput").ap()
    d_xnat = nc.dram_tensor("xnat", [128, NKT, C], f16, kind="ExternalInput").ap()
    # shared inputs
    d_wq9 = nc.dram_tensor("wq9", [C, 9, C], f16, kind="ExternalInput").ap()
    d_wk = nc.dram_tensor("wk", [C, C], f16, kind="ExternalInput").ap()
    d_wo = nc.dram_tensor("wo", [C, C], f32, kind="ExternalInput").ap()
    d_vecs = nc.dram_tensor("vecs", [C, 6], f32, kind="ExternalInput").ap()
    d_bm = nc.dram_tensor("bm", [C, C], f16, kind="ExternalInput").ap()
    d_selb = nc.dram_tensor("selb", [C, N_CORES], f32, kind="ExternalInput").ap()
    # output: this core's shard, channel-major
    d_outT = nc.dram_tensor("outT", [C, QSH], f32, kind="ExternalOutput").ap()
    if DEBUG:
        d_dbg_qc = nc.dram_tensor("dbg_qc", [C, HWPIX], f32,
                                  kind="ExternalOutput").ap()
        d_dbg_z1 = nc.dram_tensor("dbg_z1", [C, QSH], f32,
                                  kind="ExternalOutput").ap()

    with tile.TileContext(nc) as tc:
        with tc.tile_pool(name="const", bufs=1) as const, \
             tc.tile_pool(name="big", bufs=1) as big, \
             tc.tile_pool(name="epool", bufs=3) as epool, \
             tc.tile_pool(name="apool", bufs=6) as apool, \
             tc.tile_pool(name="tmp2", bufs=2) as tmp2p, \
             tc.tile_pool(name="small", bufs=2) as small, \
             tc.tile_pool(name="ps2", bufs=2, space="PSUM") as ps2, \
             tc.tile_pool(name="psd", bufs=2, space="PSUM") as psd, \
             tc.tile_pool(name="psA", bufs=2, space="PSUM") as psA, \
             tc.tile_pool(name="dram", bufs=1, space="DRAM") as dram:

            # ---------------- loads ----------------
            Wq_s = const.tile([C, 9, C], f16)
            nc.sync.dma_start(Wq_s[:], d_wq9[:])
            XpadA = big.tile([C, PADH], f16)
            nc.sync.dma_start(XpadA[:], d_xpadA[:])
            XpadB = big.tile([C, PADH], f16)
            nc.sync.dma_start(XpadB[:], d_xpadB[:])
            Xq = big.tile([C, QSH], f16)
            nc.scalar.dma_start(Xq[:], d_xqT[:])
            Xnat = big.tile([128, NKT, C], f16)
            nc.scalar.dma_start(Xnat[:], d_xnat[:])
            Wk_s = const.tile([C, C], f16)
            nc.gpsimd.dma_start(Wk_s[:], d_wk[:])
            Wo_s = const.tile([C, C], f32)
            nc.gpsimd.dma_start(Wo_s[:], d_wo[:])
            V = const.tile([C, 6], f32)
            nc.gpsimd.dma_start(V[:], d_vecs[:])
            Bb = const.tile([C, C], f16)
            nc.gpsimd.dma_start(Bb[:], d_bm[:])
            Selb = const.tile([C, N_CORES], f32)
            nc.gpsimd.dma_start(Selb[:], d_selb[:])
            eps_t = const.tile([C, 1], f32)
            nc.vector.memset(eps_t[:], EPS)

            # warm-up / act-table primer: load the one table set that holds
            # every activation this program uses (exp, ln, identity, copy)
            # up front so no mid-program table switches are ever needed.
            from concourse.hw_specs import get_activation_tables
            act_sets = list(get_activation_tables(nc.m.arch).items())
            want = next((i for i, (_, fns) in enumerate(act_sets)
                         if AF.Exp in fns and AF.Ln in fns), None)
            if want is not None:
                nc.scalar.add_instruction(mybir.InstLoadActFuncSet(
                    name=nc.get_next_instruction_name(),
                    act_func_set_id=want, ins=[], outs=[]))
            dum_x = const.tile([128, 512], f16)
            nc.vector.memset(dum_x[:], 0.0)

            def warm(n):
                for _ in range(n):
                    pw = psd.tile([128, 512], f32, tag="d")
                    nc.tensor.matmul(pw[:], dum_x[:, 0:128], dum_x[:],
                                     start=True, stop=True)

            warm(WARM_START)

            XpA = XpadA[:].rearrange("p (r c) -> p r c", r=34)
            XpB = XpadB[:].rearrange("p (r c) -> p r c", r=34)

            # rstd = exp(-0.5*ln(var+eps)) keeps ACT on the ln+exp table.
            def rstd_from_negvar(negvar, tag):
                lnv = small.tile([C, 1], f32, tag=tag + "ln")
                nc.scalar.activation(lnv[:], negvar[:], AF.Ln,
                                     scale=-1.0, bias=eps_t[:])
                r = small.tile([C, 1], f32, tag=tag + "r")
                nc.scalar.activation(r[:], lnv[:], AF.Exp, scale=-0.5)
                return r

            # a,b for y = a*z + b from gathered sums [C,2] (sum, sumsq)
            def bn_params(gsum, tot, gcol, bcol, tag):
                ms = small.tile([C, 2], f32, tag=tag + "ms")
                nc.vector.tensor_scalar_mul(ms[:], gsum[:], 1.0 / tot)
                negvar = small.tile([C, 1], f32, tag=tag + "nv")
                nc.vector.scalar_tensor_tensor(negvar[:], ms[:, 0:1],
                                               ms[:, 0:1], ms[:, 1:2],
                                               op0=OP.mult, op1=OP.subtract)
                r = rstd_from_negvar(negvar, tag)
                a = small.tile([C, 1], f32, tag=tag + "a")
                nc.vector.tensor_mul(a[:], r[:], V[:, gcol:gcol + 1])
                b = small.tile([C, 1], f32, tag=tag + "b")
                nc.vector.tensor_scalar(b[:], ms[:, 0:1], a[:], -1.0,
                                        op0=OP.mult, op1=OP.mult)
                nc.vector.tensor_add(b[:], b[:], V[:, bcol:bcol + 1])
                return a, b

            # mean/var partial accumulators -> [C,2] (sum, sumsq) partials
            def partial_sums(mv, count, tag):
                s = small.tile([C, 2], f32, tag=tag)
                nc.vector.tensor_scalar_mul(s[:, 0:1], mv[:, 0:1], count)
                m2 = small.tile([C, 1], f32, tag=tag + "m2")
                nc.vector.tensor_mul(m2[:], mv[:, 0:1], mv[:, 0:1])
                nc.vector.tensor_add(m2[:], m2[:], mv[:, 1:2])
                nc.vector.tensor_scalar_mul(s[:, 1:2], m2[:], count)
                return s

            # AllGather of a [C,2] f32 payload; returns [C,8,2] in SBUF
            def gather_stats(payload, tag):
                st_in = dram.tile([C, 2], f32, tag=tag + "i")
                st_out = dram.tile([N_CORES * C, 2], f32, tag=tag + "o",
                                   addr_space="Shared")
                nc.sync.dma_start(st_in[:], payload[:])
                nc.gpsimd.collective_compute(
                    "AllGather", mybir.AluOpType.bypass,
                    replica_groups=[list(range(N_CORES))],
                    ins=[st_in.opt()], outs=[st_out.opt()])
                g = small.tile([C, 8, 2], f32, tag=tag + "g")
                nc.sync.dma_start(
                    g[:], st_out[:].rearrange("(r c) s -> c r s", r=N_CORES))
                return g

            def reduce_ranks(g, tag):
                s = small.tile([C, 2], f32, tag=tag)
                nc.vector.tensor_reduce(
                    s[:], g[:].rearrange("c r j -> c j r"),
                    axis=AX.X, op=OP.add)
                return s

            # ---------------- CBL_Q: conv3x3 + batch stats ----------------
            # Conv output is produced directly in BLOCK-MAJOR key order:
            # chunk t covers block-row n=t, column order (m, p, q) so that
            # Qc column n*512 + m*64 + p*8 + q is pixel (8n+p, 8m+q).
            Zq = big.tile([C, 8, 512], f32)
            qstats = small.tile([C, 8, 6], f32)
            for t in range(8):
                pq = psA.tile([C, 512], f32, tag="agg")
                base = t * 8 if t < 4 else t * 8 - 32
                Xp = XpA if t < 4 else XpB
                for tap in range(9):
                    dh, dw = tap // 3 - 1, tap % 3 - 1
                    rhs = Xp[:, base + 1 + dh: base + 9 + dh,
                             1 + dw: 65 + dw].rearrange(
                                 "c p (m q) -> c m p q", m=8)
                    nc.tensor.matmul(pq[:], Wq_s[:, tap, :], rhs,
                                     start=(tap == 0), stop=(tap == 8))
                nc.vector.bn_stats(qstats[:, t, :], pq[:])
                nc.gpsimd.tensor_copy(Zq[:, t, :], pq[:])

            qmv = small.tile([C, 2], f32)
            nc.vector.bn_aggr(qmv[:], qstats[:])
            sums1 = partial_sums(qmv, float(HWPIX), "p1")

            # ---- collective 1: BN_Q stats ----
            g1t = gather_stats(sums1, "c1")
            warm(WARM_AR1)
            gsum1 = reduce_ranks(g1t, "gsum1")
            # each batch appears twice in the gathered sum
            aq, bq = bn_params(gsum1, float(HWPIX * N_CORES), 0, 1, "q")

            # q = leaky(aq*z + bq) in fp16, block-major
            Qc = big.tile([C, HWPIX], f16)
            Qv = Qc[:].rearrange("p (t f) -> p t f", f=512)
            for t in range(8):
                tmp = tmp2p.tile([C, 512], f16, tag="tmp2")
                nc.scalar.activation(tmp[:], Zq[:, t, :], AF.Identity,
                                     scale=aq[:], bias=bq[:])
                nc.vector.scalar_tensor_tensor(Qv[:, t, :], tmp[:], ALPHA,
                                               tmp[:], op0=OP.mult,
                                               op1=OP.max)

            # ---------------- attention main loop ----------------
            # software-pipelined two units deep; unit u = key tiles
            # (2u, 2u+1) of query tile u//(NKT//2)
            z1s = big.tile([C, NQT, 512], f32)
            qs1 = small.tile([C, NQT, 6], f32)
            UPQ = NKT // 2            # units per query tile
            paggs = {}
            E_t = {}
            psD_t = {}
            A_t = {}

            def emit_scores(u):
                qt = u // UPQ
                xqs = Xq[:, qt * 512:(qt + 1) * 512]
                if u % UPQ == 0:
                    pagg = psA.tile([C, 512], f32, tag="agg")
                    nc.tensor.matmul(pagg[:], Wk_s[:], xqs,
                                     start=True, stop=False)
                    paggs[qt] = pagg
                psS = ps2.tile([128, 1024], f32, tag="s")
                for j in range(2):
                    kt = 2 * (u % UPQ) + j
                    nc.tensor.matmul(psS[:, j * 512:(j + 1) * 512],
                                     Qc[:, kt * 128:(kt + 1) * 128],
                                     xqs, start=True, stop=True)
                E = epool.tile([128, 1024], f16, tag="E")
                nc.scalar.activation(E[:], psS[:], AF.Exp, scale=1.0 / RF)
                E_t[u] = E

            def emit_bbdiv(u):
                E = E_t[u]
                ds, As = [], []
                for j in range(2):
                    kt = 2 * u + j  # global unit index -> kt within qt
                    psD = psd.tile([128, 512], f32, tag="d")
                    nc.tensor.matmul(psD[:], Bb[:],
                                     E[:, j * 512:(j + 1) * 512],
                                     start=True, stop=True)
                    A = apool.tile([128, 512], f16, tag="A")
                    # one divide per engine per unit keeps both under the
                    # per-step PE budget
                    eng = nc.vector if j == 0 else nc.gpsimd
                    eng.tensor_tensor(A[:], E[:, j * 512:(j + 1) * 512],
                                      psD[:], OP.divide)
                    ds.append(psD)
                    As.append(A)
                psD_t[u] = ds
                A_t[u] = As

            def emit_agg(u):
                qt = u // UPQ
                pagg = paggs[qt]
                for j in range(2):
                    kt = 2 * (u % UPQ) + j
                    nc.tensor.matmul(pagg[:], Xnat[:, kt, :], A_t[u][j],
                                     start=False, stop=(kt == NKT - 1))
                del A_t[u]
                if u % UPQ == UPQ - 1:
                    nc.gpsimd.tensor_copy(z1s[:, qt, :], pagg[:])
                    nc.vector.bn_stats(qs1[:, qt, :], pagg[:])

            for u in range(NU):
                emit_scores(u)
                if u >= 1:
                    emit_bbdiv(u - 1)
                if u >= 3:
                    emit_agg(u - 3)
            emit_bbdiv(NU - 1)
            for u in range(NU - 3, NU):
                emit_agg(u)

            sh_mv = small.tile([C, 2], f32)
            nc.vector.bn_aggr(sh_mv[:], qs1[:])
            sums2 = partial_sums(sh_mv, float(QSH), "p2")

            if DEBUG:
                nc.sync.dma_start(d_dbg_qc[:], Qc[:])
                nc.sync.dma_start(d_dbg_z1[:],
                                  z1s[:].rearrange("c a b -> c (a b)"))

            # ---- collective 2: BN1 stats (shards are disjoint) ----
            g2t = gather_stats(sums2, "c2")
            gsum2 = reduce_ranks(g2t, "gsum2")
            TOT1 = float(B * HWPIX)
            a1, b1 = bn_params(gsum2, TOT1, 2, 3, "z")

            # e = exp(BN1(z1)) in fp16 with per-chunk f32 sums
            Ebig = big.tile([C, NQT, 512], f16)
            Ev = Ebig[:].rearrange("c t f -> c (t f)")
            z1v = z1s[:].rearrange("c t f -> c (t f)")
            esums = small.tile([C, 2], f32, tag="esums")
            for t in range(2):
                nc.scalar.activation(Ev[:, t * 1024:(t + 1) * 1024],
                                     z1v[:, t * 1024:(t + 1) * 1024], AF.Exp,
                                     scale=a1[:], bias=b1[:],
                                     accum_out=esums[:, t:t + 1])
            esh = small.tile([C, 2], f32, tag="esh")
            nc.vector.tensor_reduce(esh[:, 0:1], esums[:], axis=AX.X,
                                    op=OP.add)
            nc.vector.tensor_copy(esh[:, 1:2], esh[:, 0:1])

            # ---- collective 3: per-batch softmax sums ----
            g3t = gather_stats(esh, "c3")
            # pick this core's batch (mask is host-provided): sum over the
            # two ranks holding the same batch
            selg = small.tile([C, 8], f32)
            nc.vector.tensor_mul(selg[:], g3t[:, :, 0], Selb[:])
            sb = small.tile([C, 1], f32, tag="sb")
            nc.vector.tensor_reduce(sb[:], selg[:], axis=AX.X, op=OP.add)
            rb = small.tile([C, 1], f32, tag="rb")
            nc.vector.reciprocal(rb[:], sb[:])

            # fold the softmax normalization into the CBL_O conv weights:
            # zO = Wo^T (e * r) = (Wo * r)^T e
            WoR = const.tile([C, C], f16, tag="wor")
            nc.vector.tensor_scalar_mul(WoR[:], Wo_s[:], rb[:])

            stO = small.tile([C, 4, 6], f32)
            psO = []
            for t in range(2):
                po = ps2.tile([C, 1024], f32, tag="s")
                for j in range(2):
                    nc.tensor.matmul(po[:, j * 512:(j + 1) * 512], WoR[:],
                                     Ebig[:, 2 * t + j, :],
                                     start=True, stop=True)
                    nc.vector.bn_stats(stO[:, 2 * t + j, :],
                                       po[:, j * 512:(j + 1) * 512])
                psO.append(po)
            mvO = small.tile([C, 2], f32)
            nc.vector.bn_aggr(mvO[:], stO[:])
            sums4 = partial_sums(mvO, float(QSH), "p4")

            # ---- collective 4: BN_O stats ----
            g4t = gather_stats(sums4, "c4")
            gsum4 = reduce_ranks(g4t, "gsum4")
            aO, bO = bn_params(gsum4, TOT1, 4, 5, "o")

            for t in range(2):
                tmp = tmp2p.tile([C, 1024], f32, tag="fin")
                nc.scalar.activation(tmp[:], psO[t][:], AF.Identity,
                                     scale=aO[:], bias=bO[:])
                out_t = tmp2p.tile([C, 1024], f32, tag="fin2")
                nc.vector.scalar_tensor_tensor(out_t[:], tmp[:], ALPHA,
                                               tmp[:], op0=OP.mult,
                                               op1=OP.max)
                eng = nc.sync if t % 2 == 0 else nc.scalar
                eng.dma_start(d_outT[:, t * 1024:(t + 1) * 1024], out_t[:])

    nc.compile()
    return nc


def _get_runner():
    if "runner" in _CACHE:
        return _CACHE["runner"]
    import jax
    import numpy as np
    from jax.sharding import Mesh, PartitionSpec
    from jax.experimental.shard_map import shard_map
    from concourse import mybir
    from concourse.bass2jax import (_bass_exec_p, install_neuronx_cc_hook,
                                    partition_id_tensor)

    nc = _build_program()
    install_neuronx_cc_hook()

    in_names, out_names, out_avals, zero_outs = [], [], [], []
    partition_name = nc.partition_id_tensor.name if nc.partition_id_tensor else None
    for alloc in nc.m.functions[0].allocations:
        if not isinstance(alloc, mybir.MemoryLocationSet):
            continue
        name = alloc.memorylocations[0].name
        if alloc.kind == "ExternalInput":
            if name != partition_name:
                in_names.append(name)
        elif alloc.kind == "ExternalOutput":
            shape = tuple(alloc.tensor_shape)
            dtype = mybir.dt.np(alloc.dtype)
            out_names.append(name)
            out_avals.append(jax.core.ShapedArray(shape, dtype))
            zero_outs.append(np.zeros(shape, dtype))
    n_params = len(in_names)
    n_outs = len(out_avals)
    all_in_names = list(in_names) + list(out_names)
    if partition_name is not None:
        all_in_names.append(partition_name)

    def _body(*args):
        operands = list(args)
        if partition_name is not None:
            operands.append(partition_id_tensor())
        outs = _bass_exec_p.bind(
            *operands,
            out_avals=tuple(out_avals),
            in_names=tuple(all_in_names),
            out_names=tuple(out_names),
            lowering_input_output_aliases=(),
            sim_require_finite=True,
            sim_require_nnan=True,
            nc=nc,
        )
        return tuple(outs)

    donate = tuple(range(n_params, n_params + n_outs))
    try:
        devices = jax.devices("axon")[:N_CORES]
    except RuntimeError:
        devices = jax.devices()[:N_CORES]
    mesh = Mesh(np.asarray(devices), ("core",))
    in_specs = (PartitionSpec("core"),) * (n_params + n_outs)
    out_specs = (PartitionSpec("core"),) * n_outs
    sharded = jax.jit(
        shard_map(_body, mesh=mesh, in_specs=in_specs, out_specs=out_specs,
                  check_rep=False),
        donate_argnums=donate, keep_unused=True)

    def run(in_maps):
        per_core = [[np.asarray(m[name]) for name in in_names] for m in in_maps]
        concat_in = [np.concatenate([per_core[c][i] for c in range(N_CORES)],
                                    axis=0) for i in range(n_params)]
        concat_zeros = [np.zeros((N_CORES * z.shape[0], *z.shape[1:]), z.dtype)
                        for z in zero_outs]
        out_arrs = jax.block_until_ready(sharded(*concat_in, *concat_zeros))
        return [
            {name: np.asarray(out_arrs[i]).reshape(N_CORES, *out_avals[i].shape)[c]
             for i, name in enumerate(out_names)}
            for c in range(N_CORES)
        ]

    _CACHE["runner"] = run
    return run


def _make_blockmap():
    bm = np.zeros((C, C), np.float16)
    idx = np.arange(C)
    bm[(idx[:, None] // 64) == (idx[None, :] // 64)] = 1.0
    return bm


def kernel(x, Wq, bq, gq, btq, Wk, bk, g1, bt1, Wo, bo, go, bto):
    """Full inputs -> full output. Conv biases cancel inside training-mode
    BN (the mean subtraction removes any per-channel constant), so bq/bk/bo
    never enter the device program."""
    x = np.asarray(x, np.float32)
    run = _get_runner()

    wq9 = np.ascontiguousarray(
        np.asarray(Wq, np.float16).reshape(9, C, C).transpose(1, 0, 2))
    wk = np.ascontiguousarray(np.asarray(Wk, np.float16).reshape(C, C))
    wo = np.ascontiguousarray(np.asarray(Wo, np.float32).reshape(C, C))
    vecs = np.ascontiguousarray(np.stack(
        [np.asarray(v, np.float32) for v in (gq, btq, g1, bt1, go, bto)],
        axis=1))
    bm = _make_blockmap()

    # block-major key permutation: tile kt holds blocks (t,2j),(t,2j+1)
    # with partition index mb*64 + p*8 + q
    perm = np.arange(HWPIX).reshape(8, 8, 8, 8).transpose(0, 2, 1, 3).reshape(-1)

    in_maps = []
    for core in range(N_CORES):
        b, h = core // 2, core % 2
        xb = np.ascontiguousarray(x[b].reshape(HWPIX, C))
        xbT = xb.T  # [C, HWPIX]
        xqT = np.ascontiguousarray(xbT[:, h * QSH:(h + 1) * QSH]).astype(np.float16)
        xpadT = np.zeros((C, H + 2, W + 2), np.float16)
        xpadT[:, 1:H + 1, 1:W + 1] = xbT.reshape(C, H, W).astype(np.float16)
        xnat = np.ascontiguousarray(
            xb[perm].astype(np.float16).reshape(NKT, 128, C).transpose(1, 0, 2))
        selb = np.zeros((C, N_CORES), np.float32)
        selb[:, 2 * b] = 1.0
        selb[:, 2 * b + 1] = 1.0
        in_maps.append({
            "xqT": xqT,
            "xpadA": np.ascontiguousarray(xpadT[:, 0:34, :].reshape(C, PADH)),
            "xpadB": np.ascontiguousarray(xpadT[:, 32:66, :].reshape(C, PADH)),
            "xnat": xnat,
            "wq9": wq9, "wk": wk, "wo": wo, "vecs": vecs, "bm": bm,
            "selb": selb,
        })

    res = run(in_maps)
    out = np.empty((B, HWPIX, C), np.float32)
    for core in range(N_CORES):
        b, h = core // 2, core % 2
        out[b, h * QSH:(h + 1) * QSH, :] = res[core]["outT"].T
    return out.reshape(B, H, W, C)
